# revision 1
# baseline (speedup 1.0000x reference)
"""TRN2 Bass kernel for nn_CombinedModel (GCN x2 + DNN + head), 8 NeuronCores.

Sharding: edges sorted by dst and sharded by dst-range (12544 nodes/core).
Scatter-add is done as onehot-matmul accumulation in PSUM per 128-node block.
Gather of messages h'[src] is per-chunk indirect DMA (128 rows/instr) from an
allgathered per-layer node-feature table (bf16). dinv normalization is folded
into the tables (pre-scale by dinv[src], post-scale by dinv[dst]).
"""
import sys
sys.path.insert(0, "/opt/trn_rl_repo")
import numpy as np
import ml_dtypes

import concourse.bass as bass
import concourse.bacc as bacc
import concourse.mybir as mybir
import concourse.tile as tile
from concourse.bass_utils import run_bass_kernel_spmd
from concourse.masks import make_identity

NCORE = 8
NPC = 12544                  # nodes per core (8*12544 = 100352 >= 100000)
NTOT = NCORE * NPC
P = 128
NB = NPC // P                # 98 blocks/core
H = 64
N_NODES = 100000
BATCH = 256
DNN_IN = 768
BN_EPS = 1e-5

BF16 = mybir.dt.bfloat16
F32 = mybir.dt.float32
I32 = mybir.dt.int32
AF = mybir.ActivationFunctionType
OP = mybir.AluOpType

G_OH = 7                     # chunks per is_equal op (must divide K*NB ideally; remainder ok)


def _build(K):
    """Build the SPMD program. K = chunks per block (uniform)."""
    C = NB * K               # chunks per core per layer
    nc = bacc.Bacc("TRN2", target_bir_lowering=False, debug=False, num_devices=NCORE)

    # ---------------- I/O ----------------
    x2T_s = nc.dram_tensor("x2T_s", [P, NPC], F32, kind="ExternalInput")      # x2 shard, transposed
    dinvT = nc.dram_tensor("dinvT", [P, NB], F32, kind="ExternalInput")       # dinv[b*128+p] at [p,b]
    maskT = nc.dram_tensor("maskT", [P, NB], F32, kind="ExternalInput")       # 1.0 for real nodes
    srcpk = nc.dram_tensor("srcpk", [P, C], I32, kind="ExternalInput")        # src row of edge c*128+p
    dlpk = nc.dram_tensor("dlpk", [P, C], BF16, kind="ExternalInput")         # dst_local (255=pad)
    Wc1_d = nc.dram_tensor("Wc1_d", [P, H], F32, kind="ExternalInput")
    Wc2_d = nc.dram_tensor("Wc2_d", [H, H], BF16, kind="ExternalInput")
    bc1r = nc.dram_tensor("bc1r", [P, H], F32, kind="ExternalInput")          # bc1 replicated rows
    bc2r = nc.dram_tensor("bc2r", [P, H], F32, kind="ExternalInput")
    x1T_d = nc.dram_tensor("x1T_d", [DNN_IN, BATCH], F32, kind="ExternalInput")
    W1_d = nc.dram_tensor("W1_d", [DNN_IN, H], F32, kind="ExternalInput")
    b1r = nc.dram_tensor("b1r", [P, H], F32, kind="ExternalInput")
    gammac = nc.dram_tensor("gammac", [H, 1], F32, kind="ExternalInput")
    betac = nc.dram_tensor("betac", [H, 1], F32, kind="ExternalInput")
    Wf1_d = nc.dram_tensor("Wf1_d", [P, H], F32, kind="ExternalInput")
    bf1r = nc.dram_tensor("bf1r", [P, H], F32, kind="ExternalInput")
    Wf2_d = nc.dram_tensor("Wf2_d", [H, 1], F32, kind="ExternalInput")
    bf2r = nc.dram_tensor("bf2r", [P, 1], F32, kind="ExternalInput")
    out_d = nc.dram_tensor("out", [BATCH, 1], F32, kind="ExternalOutput")

    # internal DRAM
    h1l = nc.dram_tensor("h1l", [NPC, H], BF16)
    h1p = nc.dram_tensor("h1p", [NTOT, H], BF16, addr_space="Shared")
    h2l = nc.dram_tensor("h2l", [NPC, H], BF16)
    h2p = nc.dram_tensor("h2p", [NTOT, H], BF16, addr_space="Shared")
    gs_in = nc.dram_tensor("gs_in", [H, 1], F32)
    gs_out = nc.dram_tensor("gs_out", [H, 1], F32, addr_space="Shared")

    rg = [list(range(NCORE))]

    with tile.TileContext(nc) as tc:
        with (
            tc.tile_pool(name="cst", bufs=1) as cst,
            tc.tile_pool(name="stream", bufs=3) as stm,
            tc.tile_pool(name="gb", bufs=8) as gbp,
            tc.tile_pool(name="ohp", bufs=3) as ohp,
            tc.tile_pool(name="ev", bufs=3) as evp,
            tc.tile_pool(name="ps_acc", bufs=2, space="PSUM") as ps_acc,
            tc.tile_pool(name="ps_tp", bufs=2, space="PSUM") as ps_tp,
            tc.tile_pool(name="ps_mm2", bufs=2, space="PSUM") as ps_mm2,
            tc.tile_pool(name="ps_gs", bufs=1, space="PSUM") as ps_gs,
        ):
            # ---------- constants ----------
            iota_i = cst.tile([P, P], I32)
            nc.gpsimd.iota(iota_i[:], pattern=[[1, P]], base=0, channel_multiplier=0)
            iota_b = cst.tile([P, P], BF16)
            nc.vector.tensor_copy(iota_b[:], iota_i[:])
            ident_b = cst.tile([P, P], BF16)
            make_identity(nc, ident_b[:])
            ident_f = cst.tile([P, P], F32)
            make_identity(nc, ident_f[:])

            dinv_t = cst.tile([P, NB], F32)
            nc.sync.dma_start(out=dinv_t[:], in_=dinvT[:, :])
            mask_t = cst.tile([P, NB], F32)
            nc.sync.dma_start(out=mask_t[:], in_=maskT[:, :])
            Wc1_t = cst.tile([P, H], F32)
            nc.sync.dma_start(out=Wc1_t[:], in_=Wc1_d[:, :])
            Wc2_t = cst.tile([H, H], BF16)
            nc.sync.dma_start(out=Wc2_t[:], in_=Wc2_d[:, :])
            bc1_t = cst.tile([P, H], F32)
            nc.sync.dma_start(out=bc1_t[:], in_=bc1r[:, :])
            bc2_t = cst.tile([P, H], F32)
            nc.sync.dma_start(out=bc2_t[:], in_=bc2r[:, :])
            src_t = cst.tile([P, C], I32)
            nc.sync.dma_start(out=src_t[:], in_=srcpk[:, :])
            dl_t = cst.tile([P, C], BF16)
            nc.sync.dma_start(out=dl_t[:], in_=dlpk[:, :])

            # ---------- phase 1: h1' = dinv * (x2 @ Wc1), bf16, local shard ----------
            for b in range(NB):
                x2t = stm.tile([P, P], F32, tag="x2t")
                nc.sync.dma_start(out=x2t[:], in_=x2T_s[:, b * P:(b + 1) * P])
                ps1 = ps_mm2.tile([P, H], F32, tag="mm2")
                nc.tensor.matmul(out=ps1[:], lhsT=x2t[:], rhs=Wc1_t[:], start=True, stop=True)
                h1t = evp.tile([P, H], BF16, tag="h1t")
                nc.scalar.activation(h1t[:], ps1[:], AF.Copy, scale=dinv_t[:, b:b + 1])
                nc.sync.dma_start(out=h1l[b * P:(b + 1) * P, :], in_=h1t[:])

            nc.gpsimd.collective_compute(
                "AllGather", OP.bypass, replica_groups=rg,
                ins=[h1l.ap().opt()], outs=[h1p.ap().opt()])

            # ---------- scatter layers ----------
            def scatter_layer(table, layer):
                """Gather + onehot matmul accumulate per block; returns nothing.
                Per-block epilogues are layer-specific."""
                # onehot super-groups of G_OH chunks
                n_oh = (C + G_OH - 1) // G_OH
                oh_tiles = {}
                for g in range(n_oh):
                    c0 = g * G_OH
                    w = min(G_OH, C - c0)
                    oh = ohp.tile([P, G_OH * P], BF16, tag="oh")
                    nc.vector.tensor_tensor(
                        out=oh[:, :w * P].rearrange("p (c e) -> p c e", e=P),
                        in0=dl_t[:, c0:c0 + w].to_broadcast([P, w, P]),
                        in1=iota_b[:].rearrange("p (u e) -> p u e", u=1).to_broadcast([P, w, P]),
                        op=OP.is_equal)
                    oh_tiles[g] = oh

                for b in range(NB):
                    acc = ps_acc.tile([P, H], F32, tag="acc")
                    for k in range(K):
                        c = b * K + k
                        gb = gbp.tile([P, H], BF16, tag="gb")
                        nc.gpsimd.indirect_dma_start(
                            out=gb[:], out_offset=None, in_=table[:, :],
                            in_offset=bass.IndirectOffsetOnAxis(ap=src_t[:, c:c + 1], axis=0))
                        oh = oh_tiles[c // G_OH]
                        j = c % G_OH
                        nc.tensor.matmul(
                            out=acc[:], lhsT=oh[:, j * P:(j + 1) * P], rhs=gb[:],
                            start=(k == 0), stop=(k == K - 1))
                    if layer == 1:
                        t1 = evp.tile([P, H], F32, tag="t1")
                        nc.scalar.activation(t1[:], acc[:], AF.Copy, scale=dinv_t[:, b:b + 1])
                        g1 = evp.tile([P, H], F32, tag="g1")
                        nc.vector.tensor_tensor(out=g1[:], in0=t1[:], in1=bc1_t[:], op=OP.add)
                        nc.vector.tensor_scalar_max(g1[:], g1[:], 0.0)
                        gd = evp.tile([P, H], BF16, tag="gd")
                        nc.scalar.activation(gd[:], g1[:], AF.Copy, scale=dinv_t[:, b:b + 1])
                        tp = ps_tp.tile([H, P], BF16, tag="tp")
                        nc.tensor.transpose(out=tp[:], in_=gd[:], identity=ident_b[:])
                        gdT = evp.tile([H, P], BF16, tag="gdT")
                        nc.vector.tensor_copy(gdT[:], tp[:])
                        h2ps = ps_mm2.tile([P, H], F32, tag="mm2")
                        nc.tensor.matmul(out=h2ps[:], lhsT=gdT[:], rhs=Wc2_t[:], start=True, stop=True)
                        h2t = evp.tile([P, H], BF16, tag="h1t")
                        nc.scalar.activation(h2t[:], h2ps[:], AF.Copy)
                        nc.sync.dma_start(out=h2l[b * P:(b + 1) * P, :], in_=h2t[:])
                    else:
                        t2 = evp.tile([P, H], F32, tag="t1")
                        nc.scalar.activation(t2[:], acc[:], AF.Copy, scale=dinv_t[:, b:b + 1])
                        o2 = evp.tile([P, H], F32, tag="g1")
                        nc.vector.tensor_tensor(out=o2[:], in0=t2[:], in1=bc2_t[:], op=OP.add)
                        nc.tensor.matmul(
                            out=gs_ps[:], lhsT=o2[:], rhs=mask_t[:, b:b + 1],
                            start=(b == 0), stop=(b == NB - 1))

            scatter_layer(h1p, layer=1)
            nc.gpsimd.collective_compute(
                "AllGather", OP.bypass, replica_groups=rg,
                ins=[h2l.ap().opt()], outs=[h2p.ap().opt()])

            gs_ps = ps_gs.tile([H, 1], F32, tag="gs")
            scatter_layer(h2p, layer=2)

            gs_sb = evp.tile([H, 1], F32, tag="gs_sb")
            nc.vector.tensor_copy(gs_sb[:], gs_ps[:])
            nc.sync.dma_start(out=gs_in[:, :], in_=gs_sb[:])
            nc.gpsimd.collective_compute(
                "AllReduce", OP.add, replica_groups=rg,
                ins=[gs_in.ap().opt()], outs=[gs_out.ap().opt()])

            # ---------- head (replicated on every core) ----------
            x1_tiles, W1_tiles = [], []
            for kk in range(DNN_IN // P):
                xt = cst.tile([P, BATCH], F32, tag=f"x1_{kk}")
                nc.sync.dma_start(out=xt[:], in_=x1T_d[kk * P:(kk + 1) * P, :])
                wt = cst.tile([P, H], F32, tag=f"w1_{kk}")
                nc.sync.dma_start(out=wt[:], in_=W1_d[kk * P:(kk + 1) * P, :])
                x1_tiles.append(xt)
                W1_tiles.append(wt)
            b1_t = cst.tile([P, H], F32)
            nc.sync.dma_start(out=b1_t[:], in_=b1r[:, :])
            gam_t = cst.tile([H, 1], F32)
            nc.sync.dma_start(out=gam_t[:], in_=gammac[:, :])
            bet_t = cst.tile([H, 1], F32)
            nc.sync.dma_start(out=bet_t[:], in_=betac[:, :])
            Wf1_t = cst.tile([P, H], F32)
            nc.sync.dma_start(out=Wf1_t[:], in_=Wf1_d[:, :])
            bf1_t = cst.tile([P, H], F32)
            nc.sync.dma_start(out=bf1_t[:], in_=bf1r[:, :])
            Wf2_t = cst.tile([H, 1], F32)
            nc.sync.dma_start(out=Wf2_t[:], in_=Wf2_d[:, :])
            bf2_t = cst.tile([P, 1], F32)
            nc.sync.dma_start(out=bf2_t[:], in_=bf2r[:, :])

            dT = evp.tile([H, BATCH], F32, tag="dT")
            for half in range(2):
                dps = ps_mm2.tile([P, H], F32, tag="mm2")
                for kk in range(DNN_IN // P):
                    nc.tensor.matmul(
                        out=dps[:], lhsT=x1_tiles[kk][:, half * P:(half + 1) * P],
                        rhs=W1_tiles[kk][:], start=(kk == 0), stop=(kk == DNN_IN // P - 1))
                d_sb = evp.tile([P, H], F32, tag="d_sb")
                nc.vector.tensor_tensor(out=d_sb[:], in0=dps[:], in1=b1_t[:], op=OP.add)
                tp = ps_tp.tile([H, P], F32, tag="tp")
                nc.tensor.transpose(out=tp[:], in_=d_sb[:], identity=ident_f[:])
                nc.vector.tensor_copy(dT[:, half * P:(half + 1) * P], tp[:])
            mu = evp.tile([H, 1], F32, tag="mu")
            nc.vector.reduce_sum(mu[:], dT[:], axis=mybir.AxisListType.X)
            nc.vector.tensor_scalar_mul(mu[:], mu[:], 1.0 / BATCH)
            ctr = evp.tile([H, BATCH], F32, tag="ctr")
            nc.vector.tensor_scalar(out=ctr[:], in0=dT[:], scalar1=mu[:, :1], scalar2=None,
                                    op0=OP.subtract)
            sq = evp.tile([H, BATCH], F32, tag="sq")
            nc.vector.tensor_tensor(out=sq[:], in0=ctr[:], in1=ctr[:], op=OP.mult)
            var = evp.tile([H, 1], F32, tag="var")
            nc.vector.reduce_sum(var[:], sq[:], axis=mybir.AxisListType.X)
            nc.vector.tensor_scalar(out=var[:], in0=var[:], scalar1=1.0 / BATCH,
                                    scalar2=BN_EPS, op0=OP.mult, op1=OP.add)
            sd = evp.tile([H, 1], F32, tag="sd")
            nc.scalar.activation(sd[:], var[:], AF.Sqrt)
            rstd = evp.tile([H, 1], F32, tag="rstd")
            nc.vector.reciprocal(rstd[:], sd[:])
            sc = evp.tile([H, 1], F32, tag="sc")
            nc.vector.tensor_tensor(out=sc[:], in0=rstd[:], in1=gam_t[:], op=OP.mult)
            xT = evp.tile([P, BATCH], F32, tag="xT")
            nc.vector.tensor_scalar(out=xT[:H, :], in0=ctr[:], scalar1=sc[:, :1],
                                    scalar2=bet_t[:, :1], op0=OP.mult, op1=OP.add)
            nc.vector.tensor_scalar_max(xT[:H, :], xT[:H, :], 0.0)
            gs_t = evp.tile([H, 1], F32, tag="gs_t")
            nc.sync.dma_start(out=gs_t[:], in_=gs_out[:, :])
            gm = evp.tile([H, 1], F32, tag="gm")
            nc.scalar.activation(gm[:], gs_t[:], AF.Copy, scale=1.0 / N_NODES)
            nc.vector.tensor_copy(xT[H:P, :], gm[:, :1].to_broadcast([H, BATCH]))

            hT = evp.tile([H, BATCH], F32, tag="hT")
            for half in range(2):
                hps = ps_mm2.tile([P, H], F32, tag="mm2")
                nc.tensor.matmul(out=hps[:], lhsT=xT[:, half * P:(half + 1) * P],
                                 rhs=Wf1_t[:], start=True, stop=True)
                h_sb = evp.tile([P, H], F32, tag="d_sb")
                nc.vector.tensor_tensor(out=h_sb[:], in0=hps[:], in1=bf1_t[:], op=OP.add)
                tp = ps_tp.tile([H, P], F32, tag="tp")
                nc.tensor.transpose(out=tp[:], in_=h_sb[:], identity=ident_f[:])
                nc.vector.tensor_copy(hT[:, half * P:(half + 1) * P], tp[:])
            for half in range(2):
                yps = ps_mm2.tile([P, 1], F32, tag="mm2")
                nc.tensor.matmul(out=yps[:], lhsT=hT[:, half * P:(half + 1) * P],
                                 rhs=Wf2_t[:], start=True, stop=True)
                y_sb = evp.tile([P, 1], F32, tag="y_sb")
                nc.vector.tensor_tensor(out=y_sb[:], in0=yps[:], in1=bf2_t[:], op=OP.add)
                nc.sync.dma_start(out=out_d[half * P:(half + 1) * P, :], in_=y_sb[:])

    nc.compile()
    return nc


def _prep(inputs):
    """Host preprocessing: shard + pack edge streams."""
    ei = np.asarray(inputs["edge_index"])
    e0 = ei[0].astype(np.int64)
    e1 = ei[1].astype(np.int64)
    n = N_NODES
    loop = np.arange(n, dtype=np.int64)
    src = np.concatenate([e0, loop])
    dst = np.concatenate([e1, loop])
    deg = np.bincount(dst, minlength=NTOT).astype(np.float32)
    dinv = np.where(deg > 0, 1.0 / np.sqrt(np.maximum(deg, 1e-30)), 0.0).astype(np.float32)

    order = np.argsort(dst, kind="stable")
    src_s = src[order].astype(np.int32)
    dst_s = dst[order].astype(np.int32)
    blk = dst_s // P
    counts = np.bincount(blk, minlength=NCORE * NB)
    K = int(np.ceil(counts.max() / P))
    C = NB * K

    srcrow = np.zeros((NCORE, C * P), dtype=np.int32)
    dstloc = np.full((NCORE, C * P), 255, dtype=np.int32)
    starts = np.zeros(NCORE * NB + 1, dtype=np.int64)
    np.cumsum(counts, out=starts[1:])
    for core in range(NCORE):
        for b in range(NB):
            gidx = core * NB + b
            s, e = starts[gidx], starts[gidx + 1]
            m = e - s
            off = b * K * P
            srcrow[core, off:off + m] = src_s[s:e]
            dstloc[core, off:off + m] = dst_s[s:e] - (core * NPC + b * P)
    # pack [chunk, lane] -> [P, C]
    srcpk = srcrow.reshape(NCORE, C, P).transpose(0, 2, 1)
    dlpk = dstloc.reshape(NCORE, C, P).transpose(0, 2, 1).astype(ml_dtypes.bfloat16)
    return dinv, np.ascontiguousarray(srcpk), np.ascontiguousarray(dlpk), K


_CACHE = {}


def kernel(**inputs):
    x1 = np.asarray(inputs["x1"], np.float32)
    x2 = np.asarray(inputs["x2"], np.float32)
    W1 = np.asarray(inputs["W1"], np.float32); b1 = np.asarray(inputs["b1"], np.float32)
    gamma = np.asarray(inputs["gamma"], np.float32); beta = np.asarray(inputs["beta"], np.float32)
    Wc1 = np.asarray(inputs["Wc1"], np.float32); bc1 = np.asarray(inputs["bc1"], np.float32)
    Wc2 = np.asarray(inputs["Wc2"], np.float32); bc2 = np.asarray(inputs["bc2"], np.float32)
    Wf1 = np.asarray(inputs["Wf1"], np.float32); bf1 = np.asarray(inputs["bf1"], np.float32)
    Wf2 = np.asarray(inputs["Wf2"], np.float32); bf2 = np.asarray(inputs["bf2"], np.float32)

    dinv, srcpk, dlpk, K = _prep(inputs)

    x2p = np.zeros((NTOT, x2.shape[1]), np.float32)
    x2p[:N_NODES] = x2
    mask = np.zeros(NTOT, np.float32)
    mask[:N_NODES] = 1.0

    if K not in _CACHE:
        _CACHE[K] = _build(K)
    nc = _CACHE[K]

    rep = {
        "Wc1_d": Wc1, "Wc2_d": Wc2.astype(ml_dtypes.bfloat16),
        "bc1r": np.broadcast_to(bc1, (P, H)).copy(),
        "bc2r": np.broadcast_to(bc2, (P, H)).copy(),
        "x1T_d": np.ascontiguousarray(x1.T),
        "W1_d": W1, "b1r": np.broadcast_to(b1, (P, H)).copy(),
        "gammac": gamma[:, None].copy(), "betac": beta[:, None].copy(),
        "Wf1_d": Wf1, "bf1r": np.broadcast_to(bf1, (P, H)).copy(),
        "Wf2_d": Wf2, "bf2r": np.broadcast_to(bf2, (P, 1)).copy(),
    }
    in_maps = []
    for c in range(NCORE):
        sl = slice(c * NPC, (c + 1) * NPC)
        m = dict(rep)
        m["x2T_s"] = np.ascontiguousarray(x2p[sl].T)
        m["dinvT"] = np.ascontiguousarray(dinv[sl].reshape(NB, P).T)
        m["maskT"] = np.ascontiguousarray(mask[sl].reshape(NB, P).T)
        m["srcpk"] = srcpk[c]
        m["dlpk"] = dlpk[c]
        in_maps.append(m)

    import time
    t0 = time.time()
    res = run_bass_kernel_spmd(nc, in_maps, core_ids=list(range(NCORE)))
    kernel.last_exec_s = time.time() - t0
    return res.results[0]["out"].reshape(BATCH)



# revision 3
# speedup vs baseline: 3.5744x; 3.5744x over previous
"""TRN2 Bass kernel for nn_CombinedModel (GCN x2 + DNN + head), 8 NeuronCores.

Sharding: edges grouped by dst block and sharded by dst-range (12544 nodes/core).
Scatter-add is an onehot-matmul accumulation in PSUM per 128-node block.
Gather of messages h'[src] is per-chunk indirect DMA (128 rows/instr) from an
allgathered per-layer node-feature table (bf16). dinv normalization is folded
into the tables (pre-scale by dinv[src], post-scale by dinv[dst]).

Host->device traffic is minimized (the axon tunnel runs at ~55MB/s):
  x2 ships as fp8_e4m3 [P, NPC] (quantization error ~1e-4 after the 100K-node
  mean), edges ship as ONE i32 stream packing src | dst_local<<17 (decoded
  on device with shift/and), and the small DNN/head weights ship as one bf16
  and one f32 pack. A persistent JAX compilation cache skips the per-call
  walrus recompile that run_bass_kernel_spmd's fresh jit wrapper would do.
"""
import sys
sys.path.insert(0, "/opt/trn_rl_repo")
import os

_CC_DIR = os.path.expanduser("~/.cache/jax_bass_cc")
os.environ.setdefault("JAX_COMPILATION_CACHE_DIR", _CC_DIR)
os.environ.setdefault("JAX_PERSISTENT_CACHE_MIN_ENTRY_SIZE_BYTES", "0")
os.environ.setdefault("JAX_PERSISTENT_CACHE_MIN_COMPILE_TIME_SECS", "0")

import numpy as np
import ml_dtypes
import jax

try:
    jax.config.update("jax_compilation_cache_dir", _CC_DIR)
    jax.config.update("jax_persistent_cache_min_entry_size_bytes", 0)
    jax.config.update("jax_persistent_cache_min_compile_time_secs", 0)
except Exception:
    pass

import concourse.bass as bass
import concourse.bacc as bacc
import concourse.mybir as mybir
import concourse.tile as tile
from concourse.bass_utils import run_bass_kernel_spmd
from concourse.masks import make_identity

NCORE = 8
NPC = 12544                  # nodes per core (8*12544 = 100352 >= 100000)
NTOT = NCORE * NPC
P = 128
NB = NPC // P                # 98 blocks/core
H = 64
N_NODES = 100000
NPAD = NTOT - N_NODES        # 352 padding nodes (all zero rows in the tables)
BATCH = 256
DNN_IN = 768
KIN = DNN_IN // P            # 6 contraction chunks
BN_EPS = 1e-5

BF16 = mybir.dt.bfloat16
F32 = mybir.dt.float32
I32 = mybir.dt.int32
F8 = mybir.dt.float8e4
AF = mybir.ActivationFunctionType
OP = mybir.AluOpType

G_OH = 7                     # chunks per is_equal onehot op

# ---- bf16 pack column layout ----
BF_X1 = 0                    # KIN*[P, BATCH] chunks of x1^T
BF_W1 = BF_X1 + KIN * BATCH  # KIN*[P, H] chunks of W1
BF_WC1 = BF_W1 + KIN * H     # [P, H]
BF_WC2 = BF_WC1 + H          # [H, H] in rows 0:64
NBF = BF_WC2 + H

# ---- f32 pack column layout ----
FP_DINV = 0                  # [P, NB] dinv[b*128+p] at [p, b]
FP_BC1 = FP_DINV + NB        # [P, H] bc1 replicated rows
FP_BC2 = FP_BC1 + H
FP_B1 = FP_BC2 + H
FP_BF1 = FP_B1 + H
FP_WF1 = FP_BF1 + H          # [P, H]
FP_GAM = FP_WF1 + H          # [H, 1]
FP_BET = FP_GAM + 1          # [H, 1]
FP_WF2 = FP_BET + 1          # [H, 1]
FP_BF2 = FP_WF2 + 1          # [P, 1] bf2 replicated
FP_BC2S = FP_BF2 + 1         # [H, 1] bc2 * NPAD / N_NODES
NF = FP_BC2S + 1


def _build(K):
    """Build the SPMD program. K = chunks per block (uniform)."""
    C = NB * K               # chunks per core per layer
    nc = bacc.Bacc("TRN2", target_bir_lowering=False, debug=False, num_devices=NCORE)

    # ---------------- I/O ----------------
    x2q = nc.dram_tensor("x2q", [P, NPC], F8, kind="ExternalInput")    # x2 shard^T fp8
    epk = nc.dram_tensor("epk", [P, C], I32, kind="ExternalInput")     # src | dl<<17
    bfp = nc.dram_tensor("bfp", [P, NBF], BF16, kind="ExternalInput")
    f32p = nc.dram_tensor("f32p", [P, NF], F32, kind="ExternalInput")
    out_d = nc.dram_tensor("out", [BATCH, 1], F32, kind="ExternalOutput")

    # internal DRAM
    h1l = nc.dram_tensor("h1l", [NPC, H], BF16)
    h1p = nc.dram_tensor("h1p", [NTOT, H], BF16, addr_space="Shared")
    h2l = nc.dram_tensor("h2l", [NPC, H], BF16)
    h2p = nc.dram_tensor("h2p", [NTOT, H], BF16, addr_space="Shared")
    gs_in = nc.dram_tensor("gs_in", [H, 1], F32)
    gs_out = nc.dram_tensor("gs_out", [H, 1], F32, addr_space="Shared")

    rg = [list(range(NCORE))]

    with tile.TileContext(nc) as tc:
        with (
            tc.tile_pool(name="cst", bufs=1) as cst,
            tc.tile_pool(name="stream", bufs=3) as stm,
            tc.tile_pool(name="gb", bufs=8) as gbp,
            tc.tile_pool(name="ohp", bufs=3) as ohp,
            tc.tile_pool(name="ev", bufs=3) as evp,
            tc.tile_pool(name="ps_acc", bufs=2, space="PSUM") as ps_acc,
            tc.tile_pool(name="ps_tp", bufs=2, space="PSUM") as ps_tp,
            tc.tile_pool(name="ps_mm2", bufs=2, space="PSUM") as ps_mm2,
            tc.tile_pool(name="ps_gs", bufs=1, space="PSUM") as ps_gs,
        ):
            # ---------- constants ----------
            iota_i = cst.tile([P, P], I32)
            nc.gpsimd.iota(iota_i[:], pattern=[[1, P]], base=0, channel_multiplier=0)
            iota_b = cst.tile([P, P], BF16)
            nc.vector.tensor_copy(iota_b[:], iota_i[:])
            ident_b = cst.tile([P, P], BF16)
            make_identity(nc, ident_b[:])
            ident_f = cst.tile([P, P], F32)
            make_identity(nc, ident_f[:])
            ones_c = cst.tile([P, 1], F32)
            nc.vector.memset(ones_c[:], 1.0)

            dinv_t = cst.tile([P, NB], F32)
            nc.sync.dma_start(out=dinv_t[:], in_=f32p[:, FP_DINV:FP_DINV + NB])
            Wc1_t = cst.tile([P, H], BF16)
            nc.sync.dma_start(out=Wc1_t[:], in_=bfp[:, BF_WC1:BF_WC1 + H])
            Wc2_t = cst.tile([H, H], BF16)
            nc.sync.dma_start(out=Wc2_t[:], in_=bfp[0:H, BF_WC2:BF_WC2 + H])
            bc1_t = cst.tile([P, H], F32)
            nc.sync.dma_start(out=bc1_t[:], in_=f32p[:, FP_BC1:FP_BC1 + H])
            bc2_t = cst.tile([P, H], F32)
            nc.sync.dma_start(out=bc2_t[:], in_=f32p[:, FP_BC2:FP_BC2 + H])

            # edge stream: load packed, decode src (low 17 bits) + dl (high bits)
            epk_t = cst.tile([P, C], I32)
            nc.sync.dma_start(out=epk_t[:], in_=epk[:, :])
            src_t = cst.tile([P, C], I32)
            nc.vector.tensor_scalar(out=src_t[:], in0=epk_t[:], scalar1=0x1FFFF,
                                    scalar2=None, op0=OP.bitwise_and)
            dlh_t = cst.tile([P, C], I32)
            nc.vector.tensor_scalar(out=dlh_t[:], in0=epk_t[:], scalar1=17,
                                    scalar2=None, op0=OP.logical_shift_right)
            dl_t = cst.tile([P, C], BF16)
            nc.vector.tensor_copy(dl_t[:], dlh_t[:])

            # ---------- phase 1: h1' = dinv * (x2 @ Wc1), bf16, local shard ----------
            for b in range(NB):
                x2t8 = stm.tile([P, P], F8, tag="x2t8")
                nc.sync.dma_start(out=x2t8[:], in_=x2q[:, b * P:(b + 1) * P])
                x2t = stm.tile([P, P], BF16, tag="x2t")
                nc.vector.tensor_copy(x2t[:], x2t8[:])
                ps1 = ps_mm2.tile([P, H], F32, tag="mm2")
                nc.tensor.matmul(out=ps1[:], lhsT=x2t[:], rhs=Wc1_t[:], start=True, stop=True)
                h1t = evp.tile([P, H], BF16, tag="h1t")
                nc.scalar.activation(h1t[:], ps1[:], AF.Copy, scale=dinv_t[:, b:b + 1])
                nc.sync.dma_start(out=h1l[b * P:(b + 1) * P, :], in_=h1t[:])

            nc.gpsimd.collective_compute(
                "AllGather", OP.bypass, replica_groups=rg,
                ins=[h1l.ap().opt()], outs=[h1p.ap().opt()])

            # ---------- scatter layers ----------
            def scatter_layer(table, layer):
                """Gather + onehot matmul accumulate per block, layer-specific epilogue."""
                n_oh = (C + G_OH - 1) // G_OH
                oh_tiles = {}
                for g in range(n_oh):
                    c0 = g * G_OH
                    w = min(G_OH, C - c0)
                    oh = ohp.tile([P, G_OH * P], BF16, tag="oh")
                    nc.vector.tensor_tensor(
                        out=oh[:, :w * P].rearrange("p (c e) -> p c e", e=P),
                        in0=dl_t[:, c0:c0 + w].to_broadcast([P, w, P]),
                        in1=iota_b[:].rearrange("p (u e) -> p u e", u=1).to_broadcast([P, w, P]),
                        op=OP.is_equal)
                    oh_tiles[g] = oh

                for b in range(NB):
                    acc = ps_acc.tile([P, H], F32, tag="acc")
                    for k in range(K):
                        c = b * K + k
                        gb = gbp.tile([P, H], BF16, tag="gb")
                        nc.gpsimd.indirect_dma_start(
                            out=gb[:], out_offset=None, in_=table[:, :],
                            in_offset=bass.IndirectOffsetOnAxis(ap=src_t[:, c:c + 1], axis=0))
                        oh = oh_tiles[c // G_OH]
                        j = c % G_OH
                        nc.tensor.matmul(
                            out=acc[:], lhsT=oh[:, j * P:(j + 1) * P], rhs=gb[:],
                            start=(k == 0), stop=(k == K - 1))
                    if layer == 1:
                        t1 = evp.tile([P, H], F32, tag="t1")
                        nc.scalar.activation(t1[:], acc[:], AF.Copy, scale=dinv_t[:, b:b + 1])
                        g1 = evp.tile([P, H], F32, tag="g1")
                        nc.vector.tensor_tensor(out=g1[:], in0=t1[:], in1=bc1_t[:], op=OP.add)
                        nc.vector.tensor_scalar_max(g1[:], g1[:], 0.0)
                        gd = evp.tile([P, H], BF16, tag="gd")
                        nc.scalar.activation(gd[:], g1[:], AF.Copy, scale=dinv_t[:, b:b + 1])
                        tp = ps_tp.tile([H, P], BF16, tag="tp")
                        nc.tensor.transpose(out=tp[:], in_=gd[:], identity=ident_b[:])
                        gdT = evp.tile([H, P], BF16, tag="gdT")
                        nc.vector.tensor_copy(gdT[:], tp[:])
                        h2ps = ps_mm2.tile([P, H], F32, tag="mm2")
                        nc.tensor.matmul(out=h2ps[:], lhsT=gdT[:], rhs=Wc2_t[:], start=True, stop=True)
                        h2t = evp.tile([P, H], BF16, tag="h1t")
                        nc.scalar.activation(h2t[:], h2ps[:], AF.Copy)
                        nc.sync.dma_start(out=h2l[b * P:(b + 1) * P, :], in_=h2t[:])
                    else:
                        t2 = evp.tile([P, H], F32, tag="t1")
                        nc.scalar.activation(t2[:], acc[:], AF.Copy, scale=dinv_t[:, b:b + 1])
                        o2 = evp.tile([P, H], F32, tag="g1")
                        nc.vector.tensor_tensor(out=o2[:], in0=t2[:], in1=bc2_t[:], op=OP.add)
                        nc.tensor.matmul(
                            out=gs_ps[:], lhsT=o2[:], rhs=ones_c[:, 0:1],
                            start=(b == 0), stop=(b == NB - 1))

            scatter_layer(h1p, layer=1)
            nc.gpsimd.collective_compute(
                "AllGather", OP.bypass, replica_groups=rg,
                ins=[h2l.ap().opt()], outs=[h2p.ap().opt()])

            gs_ps = ps_gs.tile([H, 1], F32, tag="gs")
            scatter_layer(h2p, layer=2)

            gs_sb = evp.tile([H, 1], F32, tag="gs_sb")
            nc.vector.tensor_copy(gs_sb[:], gs_ps[:])
            nc.sync.dma_start(out=gs_in[:, :], in_=gs_sb[:])
            nc.gpsimd.collective_compute(
                "AllReduce", OP.add, replica_groups=rg,
                ins=[gs_in.ap().opt()], outs=[gs_out.ap().opt()])

            # ---------- head (replicated on every core) ----------
            x1_tiles, W1_tiles = [], []
            for kk in range(KIN):
                xt = cst.tile([P, BATCH], BF16, tag=f"x1_{kk}")
                nc.sync.dma_start(out=xt[:], in_=bfp[:, BF_X1 + kk * BATCH:BF_X1 + (kk + 1) * BATCH])
                wt = cst.tile([P, H], BF16, tag=f"w1_{kk}")
                nc.sync.dma_start(out=wt[:], in_=bfp[:, BF_W1 + kk * H:BF_W1 + (kk + 1) * H])
                x1_tiles.append(xt)
                W1_tiles.append(wt)
            b1_t = cst.tile([P, H], F32)
            nc.sync.dma_start(out=b1_t[:], in_=f32p[:, FP_B1:FP_B1 + H])
            gam_t = cst.tile([H, 1], F32)
            nc.sync.dma_start(out=gam_t[:], in_=f32p[0:H, FP_GAM:FP_GAM + 1])
            bet_t = cst.tile([H, 1], F32)
            nc.sync.dma_start(out=bet_t[:], in_=f32p[0:H, FP_BET:FP_BET + 1])
            Wf1_t = cst.tile([P, H], F32)
            nc.sync.dma_start(out=Wf1_t[:], in_=f32p[:, FP_WF1:FP_WF1 + H])
            bf1_t = cst.tile([P, H], F32)
            nc.sync.dma_start(out=bf1_t[:], in_=f32p[:, FP_BF1:FP_BF1 + H])
            Wf2_t = cst.tile([H, 1], F32)
            nc.sync.dma_start(out=Wf2_t[:], in_=f32p[0:H, FP_WF2:FP_WF2 + 1])
            bf2_t = cst.tile([P, 1], F32)
            nc.sync.dma_start(out=bf2_t[:], in_=f32p[:, FP_BF2:FP_BF2 + 1])
            bc2s_t = cst.tile([H, 1], F32)
            nc.sync.dma_start(out=bc2s_t[:], in_=f32p[0:H, FP_BC2S:FP_BC2S + 1])

            dT = evp.tile([H, BATCH], F32, tag="dT")
            for half in range(2):
                dps = ps_mm2.tile([P, H], F32, tag="mm2")
                for kk in range(KIN):
                    nc.tensor.matmul(
                        out=dps[:], lhsT=x1_tiles[kk][:, half * P:(half + 1) * P],
                        rhs=W1_tiles[kk][:], start=(kk == 0), stop=(kk == KIN - 1))
                d_sb = evp.tile([P, H], F32, tag="d_sb")
                nc.vector.tensor_tensor(out=d_sb[:], in0=dps[:], in1=b1_t[:], op=OP.add)
                tp = ps_tp.tile([H, P], F32, tag="tp")
                nc.tensor.transpose(out=tp[:], in_=d_sb[:], identity=ident_f[:])
                nc.vector.tensor_copy(dT[:, half * P:(half + 1) * P], tp[:])
            mu = evp.tile([H, 1], F32, tag="mu")
            nc.vector.reduce_sum(mu[:], dT[:], axis=mybir.AxisListType.X)
            nc.vector.tensor_scalar_mul(mu[:], mu[:], 1.0 / BATCH)
            ctr = evp.tile([H, BATCH], F32, tag="ctr")
            nc.vector.tensor_scalar(out=ctr[:], in0=dT[:], scalar1=mu[:, :1], scalar2=None,
                                    op0=OP.subtract)
            sq = evp.tile([H, BATCH], F32, tag="sq")
            nc.vector.tensor_tensor(out=sq[:], in0=ctr[:], in1=ctr[:], op=OP.mult)
            var = evp.tile([H, 1], F32, tag="var")
            nc.vector.reduce_sum(var[:], sq[:], axis=mybir.AxisListType.X)
            nc.vector.tensor_scalar(out=var[:], in0=var[:], scalar1=1.0 / BATCH,
                                    scalar2=BN_EPS, op0=OP.mult, op1=OP.add)
            sd = evp.tile([H, 1], F32, tag="sd")
            nc.scalar.activation(sd[:], var[:], AF.Sqrt)
            rstd = evp.tile([H, 1], F32, tag="rstd")
            nc.vector.reciprocal(rstd[:], sd[:])
            sc = evp.tile([H, 1], F32, tag="sc")
            nc.vector.tensor_tensor(out=sc[:], in0=rstd[:], in1=gam_t[:], op=OP.mult)
            xT = evp.tile([P, BATCH], F32, tag="xT")
            nc.vector.tensor_scalar(out=xT[:H, :], in0=ctr[:], scalar1=sc[:, :1],
                                    scalar2=bet_t[:, :1], op0=OP.mult, op1=OP.add)
            nc.vector.tensor_scalar_max(xT[:H, :], xT[:H, :], 0.0)
            gs_t = evp.tile([H, 1], F32, tag="gs_t")
            nc.sync.dma_start(out=gs_t[:], in_=gs_out[:, :])
            gm = evp.tile([H, 1], F32, tag="gm")
            nc.vector.tensor_scalar(out=gm[:], in0=gs_t[:], scalar1=1.0 / N_NODES,
                                    scalar2=None, op0=OP.mult)
            nc.vector.tensor_tensor(out=gm[:], in0=gm[:], in1=bc2s_t[:], op=OP.subtract)
            nc.vector.tensor_copy(xT[H:P, :], gm[:, :1].to_broadcast([H, BATCH]))

            hT = evp.tile([H, BATCH], F32, tag="hT")
            for half in range(2):
                hps = ps_mm2.tile([P, H], F32, tag="mm2")
                nc.tensor.matmul(out=hps[:], lhsT=xT[:, half * P:(half + 1) * P],
                                 rhs=Wf1_t[:], start=True, stop=True)
                h_sb = evp.tile([P, H], F32, tag="d_sb")
                nc.vector.tensor_tensor(out=h_sb[:], in0=hps[:], in1=bf1_t[:], op=OP.add)
                tp = ps_tp.tile([H, P], F32, tag="tp")
                nc.tensor.transpose(out=tp[:], in_=h_sb[:], identity=ident_f[:])
                nc.vector.tensor_copy(hT[:, half * P:(half + 1) * P], tp[:])
            for half in range(2):
                yps = ps_mm2.tile([P, 1], F32, tag="mm2")
                nc.tensor.matmul(out=yps[:], lhsT=hT[:, half * P:(half + 1) * P],
                                 rhs=Wf2_t[:], start=True, stop=True)
                y_sb = evp.tile([P, 1], F32, tag="y_sb")
                nc.vector.tensor_tensor(out=y_sb[:], in0=yps[:], in1=bf2_t[:], op=OP.add)
                nc.sync.dma_start(out=out_d[half * P:(half + 1) * P, :], in_=y_sb[:])

    nc.compile()
    return nc


def _prep(inputs):
    """Host preprocessing: degree norm + packed per-core edge streams."""
    ei = np.asarray(inputs["edge_index"])
    loop = np.arange(N_NODES, dtype=np.int64)
    src = np.concatenate([ei[0].astype(np.int64), loop]).astype(np.int32)
    dst = np.concatenate([ei[1].astype(np.int64), loop]).astype(np.int32)
    E = src.shape[0]
    deg = np.bincount(dst, minlength=NTOT).astype(np.float32)
    dinv = np.where(deg > 0, 1.0 / np.sqrt(np.maximum(deg, 1e-30)), 0.0).astype(np.float32)

    blk = (dst >> 7).astype(np.uint16)
    order = np.argsort(blk, kind="stable")       # radix sort on u16 keys
    src_s = src[order]
    dst_s = dst[order]
    blk_s = blk[order].astype(np.int64)
    counts = np.bincount(blk_s, minlength=NCORE * NB)
    K = int(np.ceil(counts.max() / P))
    C = NB * K
    starts = np.zeros(NCORE * NB + 1, dtype=np.int64)
    np.cumsum(counts, out=starts[1:])
    rank = np.arange(E, dtype=np.int64) - starts[blk_s]
    core = blk_s // NB
    b = blk_s - core * NB
    flat = b * (K * P) + rank
    chunk = flat // P
    lane = flat - chunk * P
    val = src_s | ((dst_s & 127) << 17)
    epk = np.full((NCORE, P, C), (255 << 17) | (NTOT - 1), dtype=np.int32)
    idx = core * (P * C) + lane * C + chunk
    epk.reshape(-1)[idx] = val
    return dinv, epk, K


_CACHE = {}


def kernel(**inputs):
    x1 = np.asarray(inputs["x1"], np.float32)
    x2 = np.asarray(inputs["x2"], np.float32)
    W1 = np.asarray(inputs["W1"], np.float32); b1 = np.asarray(inputs["b1"], np.float32)
    gamma = np.asarray(inputs["gamma"], np.float32); beta = np.asarray(inputs["beta"], np.float32)
    Wc1 = np.asarray(inputs["Wc1"], np.float32); bc1 = np.asarray(inputs["bc1"], np.float32)
    Wc2 = np.asarray(inputs["Wc2"], np.float32); bc2 = np.asarray(inputs["bc2"], np.float32)
    Wf1 = np.asarray(inputs["Wf1"], np.float32); bf1 = np.asarray(inputs["bf1"], np.float32)
    Wf2 = np.asarray(inputs["Wf2"], np.float32); bf2 = np.asarray(inputs["bf2"], np.float32)

    dinv, epk, K = _prep(inputs)

    x2p = np.zeros((NTOT, x2.shape[1]), np.float32)
    x2p[:N_NODES] = x2

    if K not in _CACHE:
        _CACHE[K] = _build(K)
    nc = _CACHE[K]

    bf = ml_dtypes.bfloat16
    bfp = np.zeros((P, NBF), bf)
    x1T = np.ascontiguousarray(x1.T)             # [768, 256]
    bfp[:, BF_X1:BF_X1 + KIN * BATCH] = x1T.reshape(KIN, P, BATCH).transpose(1, 0, 2).reshape(P, KIN * BATCH).astype(bf)
    bfp[:, BF_W1:BF_W1 + KIN * H] = W1.reshape(KIN, P, H).transpose(1, 0, 2).reshape(P, KIN * H).astype(bf)
    bfp[:, BF_WC1:BF_WC1 + H] = Wc1.astype(bf)
    bfp[0:H, BF_WC2:BF_WC2 + H] = Wc2.astype(bf)

    f32p = np.zeros((P, NF), np.float32)
    f32p[:, FP_BC1:FP_BC1 + H] = bc1
    f32p[:, FP_BC2:FP_BC2 + H] = bc2
    f32p[:, FP_B1:FP_B1 + H] = b1
    f32p[:, FP_BF1:FP_BF1 + H] = bf1
    f32p[:, FP_WF1:FP_WF1 + H] = Wf1
    f32p[0:H, FP_GAM] = gamma
    f32p[0:H, FP_BET] = beta
    f32p[0:H, FP_WF2] = Wf2[:, 0]
    f32p[:, FP_BF2] = bf2[0]
    f32p[0:H, FP_BC2S] = bc2 * (float(NPAD) / N_NODES)

    f8 = ml_dtypes.float8_e4m3
    in_maps = []
    for c in range(NCORE):
        sl = slice(c * NPC, (c + 1) * NPC)
        fp = f32p.copy()
        fp[:, FP_DINV:FP_DINV + NB] = dinv[sl].reshape(NB, P).T
        in_maps.append({
            "x2q": np.ascontiguousarray(x2p[sl].T).astype(f8),
            "epk": epk[c],
            "bfp": bfp,
            "f32p": fp,
        })

    import time
    t0 = time.time()
    res = run_bass_kernel_spmd(nc, in_maps, core_ids=list(range(NCORE)))
    kernel.last_exec_s = time.time() - t0
    return res.results[0]["out"].reshape(BATCH)


# revision 5
# speedup vs baseline: 4.0488x; 1.1327x over previous
"""TRN2 Bass kernel for nn_CombinedModel (GCN x2 + DNN + head), 8 NeuronCores.

Sharding: edges grouped by dst block and sharded by dst-range (12544 nodes/core).
Scatter-add is an onehot-matmul accumulation in PSUM per 128-node block.
Gather of messages h'[src] is per-chunk indirect DMA (128 rows/instr) from an
allgathered per-layer node-feature table (bf16). dinv normalization is folded
into the tables (pre-scale by dinv[src], post-scale by dinv[dst]).

Host->device traffic is minimized (the axon tunnel runs at ~55MB/s, with
~18ms fixed latency per input tensor):
  - x2 ships as fp8_e4m3 [P, NPC] (quantization error ~1e-4 after the
    100K-node mean),
  - edges ship as ONE u8 stream [P, 3C]: three bytes of src | dst_local<<17
    (dst_local < 128, so the word fits 24 bits), reassembled + decoded on
    device with shift/and,
  - x1 is batch-sharded (32 rows/core); per-core d = x1_c @ W1 is
    AllGathered (8KB) before BatchNorm,
  - small weights ship as one bf16 pack and one f32 pack; biases ship as
    single rows (broadcast on device via a rank-1 ones matmul) or as [H,1]
    columns applied after the transpose.
A persistent JAX compilation cache skips the per-call walrus recompile that
run_bass_kernel_spmd's fresh jit wrapper would otherwise do.
"""
import sys
sys.path.insert(0, "/opt/trn_rl_repo")
import os

_CC_DIR = os.path.expanduser("~/.cache/jax_bass_cc")
os.environ.setdefault("JAX_COMPILATION_CACHE_DIR", _CC_DIR)
os.environ.setdefault("JAX_PERSISTENT_CACHE_MIN_ENTRY_SIZE_BYTES", "0")
os.environ.setdefault("JAX_PERSISTENT_CACHE_MIN_COMPILE_TIME_SECS", "0")

import numpy as np
import ml_dtypes
import jax

try:
    jax.config.update("jax_compilation_cache_dir", _CC_DIR)
    jax.config.update("jax_persistent_cache_min_entry_size_bytes", 0)
    jax.config.update("jax_persistent_cache_min_compile_time_secs", 0)
except Exception:
    pass

import concourse.bass as bass
import concourse.bacc as bacc
import concourse.mybir as mybir
import concourse.tile as tile
from concourse.bass_utils import run_bass_kernel_spmd
from concourse.masks import make_identity

NCORE = 8
NPC = 12544                  # nodes per core (8*12544 = 100352 >= 100000)
NTOT = NCORE * NPC
P = 128
NB = NPC // P                # 98 blocks/core
H = 64
N_NODES = 100000
NPAD = NTOT - N_NODES        # 352 padding nodes (all zero rows in the tables)
BATCH = 256
BPC = BATCH // NCORE         # 32 DNN rows per core
DNN_IN = 768
KIN = DNN_IN // P            # 6 contraction chunks
BN_EPS = 1e-5

BF16 = mybir.dt.bfloat16
F32 = mybir.dt.float32
I32 = mybir.dt.int32
F8 = mybir.dt.float8e4
U8 = mybir.dt.uint8
AF = mybir.ActivationFunctionType
OP = mybir.AluOpType

G_OH = 7                     # chunks per is_equal onehot op

# ---- bf16 pack column layout ----
BF_X1 = 0                    # KIN*[P, BPC] chunks of this core's x1^T columns
BF_W1 = BF_X1 + KIN * BPC    # KIN*[P, H] chunks of W1
BF_WC1 = BF_W1 + KIN * H     # [P, H]
BF_WC2 = BF_WC1 + H          # [H, H] in rows 0:64
NBF = BF_WC2 + H

# ---- f32 pack column layout ----
FP_DINV = 0                  # [P, NB] dinv[b*128+p] at [p, b]
FP_WF1 = FP_DINV + NB        # [P, H]
FP_BCR = FP_WF1 + H          # 64-col block: row0=bc1, row1=bc2, row2=bf1
FP_GAM = FP_BCR + H          # [H, 1] columns from here on
FP_BET = FP_GAM + 1
FP_WF2 = FP_BET + 1
FP_BF2 = FP_WF2 + 1          # [P, 1] bf2 replicated
FP_BC2S = FP_BF2 + 1         # [H, 1] bc2 * NPAD / N_NODES
FP_B1C = FP_BC2S + 1         # [H, 1] b1 as column
FP_BC2C = FP_B1C + 1         # [H, 1] bc2 as column (for layer-2 epilogue)
NF = FP_BC2C + 1


def _build(K):
    """Build the SPMD program. K = chunks per block (uniform)."""
    C = NB * K               # chunks per core per layer
    nc = bacc.Bacc("TRN2", target_bir_lowering=False, debug=False, num_devices=NCORE)

    # ---------------- I/O ----------------
    x2q = nc.dram_tensor("x2q", [P, NPC], F8, kind="ExternalInput")    # x2 shard^T fp8
    e8 = nc.dram_tensor("e8", [P, 3 * C], U8, kind="ExternalInput")    # 3-byte edge words
    bfp = nc.dram_tensor("bfp", [P, NBF], BF16, kind="ExternalInput")
    f32p = nc.dram_tensor("f32p", [P, NF], F32, kind="ExternalInput")
    out_d = nc.dram_tensor("out", [BATCH, 1], F32, kind="ExternalOutput")

    # internal DRAM
    h1l = nc.dram_tensor("h1l", [NPC, H], BF16)
    h1p = nc.dram_tensor("h1p", [NTOT, H], BF16, addr_space="Shared")
    h2l = nc.dram_tensor("h2l", [NPC, H], BF16)
    h2p = nc.dram_tensor("h2p", [NTOT, H], BF16, addr_space="Shared")
    dl_d = nc.dram_tensor("dl_d", [BPC, H], F32)
    dg_d = nc.dram_tensor("dg_d", [BATCH, H], F32, addr_space="Shared")
    gs_in = nc.dram_tensor("gs_in", [H, 1], F32)
    gs_out = nc.dram_tensor("gs_out", [H, 1], F32, addr_space="Shared")

    rg = [list(range(NCORE))]

    with tile.TileContext(nc) as tc:
        with (
            tc.tile_pool(name="cst", bufs=1) as cst,
            tc.tile_pool(name="scr", bufs=4) as scr,
            tc.tile_pool(name="stream", bufs=3) as stm,
            tc.tile_pool(name="gb", bufs=8) as gbp,
            tc.tile_pool(name="ohp", bufs=3) as ohp,
            tc.tile_pool(name="ev", bufs=3) as evp,
            tc.tile_pool(name="ps_acc", bufs=2, space="PSUM") as ps_acc,
            tc.tile_pool(name="ps_tp", bufs=2, space="PSUM") as ps_tp,
            tc.tile_pool(name="ps_mm2", bufs=2, space="PSUM") as ps_mm2,
            tc.tile_pool(name="ps_gs", bufs=1, space="PSUM") as ps_gs,
        ):
            # ---------- constants ----------
            iota_i = cst.tile([P, P], I32)
            nc.gpsimd.iota(iota_i[:], pattern=[[1, P]], base=0, channel_multiplier=0)
            iota_b = cst.tile([P, P], BF16)
            nc.vector.tensor_copy(iota_b[:], iota_i[:])
            ident_b = cst.tile([P, P], BF16)
            make_identity(nc, ident_b[:])
            ident_f = cst.tile([P, P], F32)
            make_identity(nc, ident_f[:])
            ones_c = cst.tile([P, 1], F32)
            nc.vector.memset(ones_c[:], 1.0)
            ones_r = cst.tile([1, P], F32)
            nc.vector.memset(ones_r[:], 1.0)

            dinv_t = cst.tile([P, NB], F32)
            nc.sync.dma_start(out=dinv_t[:], in_=f32p[:, FP_DINV:FP_DINV + NB])
            Wc1_t = cst.tile([P, H], BF16)
            nc.sync.dma_start(out=Wc1_t[:], in_=bfp[:, BF_WC1:BF_WC1 + H])
            Wc2_t = cst.tile([H, H], BF16)
            nc.sync.dma_start(out=Wc2_t[:], in_=bfp[0:H, BF_WC2:BF_WC2 + H])

            # broadcast single-row biases to [P, H] via rank-1 ones matmul
            bc1_row = cst.tile([1, H], F32)
            nc.sync.dma_start(out=bc1_row[:], in_=f32p[0:1, FP_BCR:FP_BCR + H])
            bc2_row = cst.tile([1, H], F32)
            nc.sync.dma_start(out=bc2_row[:], in_=f32p[1:2, FP_BCR:FP_BCR + H])
            bf1_row = cst.tile([1, H], F32)
            nc.sync.dma_start(out=bf1_row[:], in_=f32p[2:3, FP_BCR:FP_BCR + H])
            bc1_t = cst.tile([P, H], F32)
            bc2_t = cst.tile([P, H], F32)
            bf1_t = cst.tile([P, H], F32)
            for row, dstt in ((bc1_row, bc1_t), (bc2_row, bc2_t), (bf1_row, bf1_t)):
                bps = ps_tp.tile([P, H], F32, tag="tp")
                nc.tensor.matmul(out=bps[:], lhsT=ones_r[:], rhs=row[:], start=True, stop=True)
                nc.vector.tensor_copy(dstt[:], bps[:])

            # edge stream: reassemble 3-byte words, decode src / dst_local
            e8_t = cst.tile([P, 3 * C], U8)
            nc.sync.dma_start(out=e8_t[:], in_=e8[:, :])
            b0_t = scr.tile([P, C], I32, tag="eb")
            nc.vector.tensor_copy(b0_t[:], e8_t[:, 0:C])
            b1_t = scr.tile([P, C], I32, tag="eb")
            nc.vector.tensor_copy(b1_t[:], e8_t[:, C:2 * C])
            b2_t = scr.tile([P, C], I32, tag="eb")
            nc.vector.tensor_copy(b2_t[:], e8_t[:, 2 * C:3 * C])
            v_t = scr.tile([P, C], I32, tag="eb")
            nc.vector.tensor_scalar(out=v_t[:], in0=b1_t[:], scalar1=8,
                                    scalar2=None, op0=OP.logical_shift_left)
            nc.vector.tensor_tensor(out=v_t[:], in0=v_t[:], in1=b0_t[:], op=OP.add)
            nc.vector.tensor_scalar(out=b2_t[:], in0=b2_t[:], scalar1=16,
                                    scalar2=None, op0=OP.logical_shift_left)
            nc.vector.tensor_tensor(out=v_t[:], in0=v_t[:], in1=b2_t[:], op=OP.add)
            src_t = cst.tile([P, C], I32)
            nc.vector.tensor_scalar(out=src_t[:], in0=v_t[:], scalar1=0x1FFFF,
                                    scalar2=None, op0=OP.bitwise_and)
            dlh_t = scr.tile([P, C], I32, tag="eb")
            nc.vector.tensor_scalar(out=dlh_t[:], in0=v_t[:], scalar1=17,
                                    scalar2=None, op0=OP.logical_shift_right)
            dl_t = cst.tile([P, C], BF16)
            nc.vector.tensor_copy(dl_t[:], dlh_t[:])

            # ---------- DNN branch: d_local = x1_c @ W1, allgather ----------
            x1_tiles, W1_tiles = [], []
            for kk in range(KIN):
                xt = cst.tile([P, BPC], BF16, tag=f"x1_{kk}")
                nc.sync.dma_start(out=xt[:], in_=bfp[:, BF_X1 + kk * BPC:BF_X1 + (kk + 1) * BPC])
                wt = cst.tile([P, H], BF16, tag=f"w1_{kk}")
                nc.sync.dma_start(out=wt[:], in_=bfp[:, BF_W1 + kk * H:BF_W1 + (kk + 1) * H])
                x1_tiles.append(xt)
                W1_tiles.append(wt)
            dps = ps_mm2.tile([BPC, H], F32, tag="mm2")
            for kk in range(KIN):
                nc.tensor.matmul(out=dps[:], lhsT=x1_tiles[kk][:], rhs=W1_tiles[kk][:],
                                 start=(kk == 0), stop=(kk == KIN - 1))
            d_sb = evp.tile([BPC, H], F32, tag="d_sb")
            nc.vector.tensor_copy(d_sb[:], dps[:])
            nc.sync.dma_start(out=dl_d[:, :], in_=d_sb[:])
            nc.gpsimd.collective_compute(
                "AllGather", OP.bypass, replica_groups=rg,
                ins=[dl_d.ap().opt()], outs=[dg_d.ap().opt()])

            # ---------- phase 1: h1' = dinv * (x2 @ Wc1), bf16, local shard ----------
            for b in range(NB):
                x2t8 = stm.tile([P, P], F8, tag="x2t8")
                nc.sync.dma_start(out=x2t8[:], in_=x2q[:, b * P:(b + 1) * P])
                x2t = stm.tile([P, P], BF16, tag="x2t")
                nc.vector.tensor_copy(x2t[:], x2t8[:])
                ps1 = ps_mm2.tile([P, H], F32, tag="mm2")
                nc.tensor.matmul(out=ps1[:], lhsT=x2t[:], rhs=Wc1_t[:], start=True, stop=True)
                h1t = evp.tile([P, H], BF16, tag="h1t")
                nc.scalar.activation(h1t[:], ps1[:], AF.Copy, scale=dinv_t[:, b:b + 1])
                nc.sync.dma_start(out=h1l[b * P:(b + 1) * P, :], in_=h1t[:])

            nc.gpsimd.collective_compute(
                "AllGather", OP.bypass, replica_groups=rg,
                ins=[h1l.ap().opt()], outs=[h1p.ap().opt()])

            # ---------- scatter layers ----------
            def scatter_layer(table, layer):
                """Gather + onehot matmul accumulate per block, layer-specific epilogue."""
                n_oh = (C + G_OH - 1) // G_OH
                oh_tiles = {}
                for g in range(n_oh):
                    c0 = g * G_OH
                    w = min(G_OH, C - c0)
                    oh = ohp.tile([P, G_OH * P], BF16, tag="oh")
                    nc.vector.tensor_tensor(
                        out=oh[:, :w * P].rearrange("p (c e) -> p c e", e=P),
                        in0=dl_t[:, c0:c0 + w].to_broadcast([P, w, P]),
                        in1=iota_b[:].rearrange("p (u e) -> p u e", u=1).to_broadcast([P, w, P]),
                        op=OP.is_equal)
                    oh_tiles[g] = oh

                for b in range(NB):
                    acc = ps_acc.tile([P, H], F32, tag="acc")
                    for k in range(K):
                        c = b * K + k
                        gb = gbp.tile([P, H], BF16, tag="gb")
                        nc.gpsimd.indirect_dma_start(
                            out=gb[:], out_offset=None, in_=table[:, :],
                            in_offset=bass.IndirectOffsetOnAxis(ap=src_t[:, c:c + 1], axis=0))
                        oh = oh_tiles[c // G_OH]
                        j = c % G_OH
                        nc.tensor.matmul(
                            out=acc[:], lhsT=oh[:, j * P:(j + 1) * P], rhs=gb[:],
                            start=(k == 0), stop=(k == K - 1))
                    if layer == 1:
                        t1 = evp.tile([P, H], F32, tag="t1")
                        nc.scalar.activation(t1[:], acc[:], AF.Copy, scale=dinv_t[:, b:b + 1])
                        g1 = evp.tile([P, H], F32, tag="g1")
                        nc.vector.tensor_tensor(out=g1[:], in0=t1[:], in1=bc1_t[:], op=OP.add)
                        nc.vector.tensor_scalar_max(g1[:], g1[:], 0.0)
                        gd = evp.tile([P, H], BF16, tag="gd")
                        nc.scalar.activation(gd[:], g1[:], AF.Copy, scale=dinv_t[:, b:b + 1])
                        tp = ps_tp.tile([H, P], BF16, tag="tp")
                        nc.tensor.transpose(out=tp[:], in_=gd[:], identity=ident_b[:])
                        gdT = evp.tile([H, P], BF16, tag="gdT")
                        nc.vector.tensor_copy(gdT[:], tp[:])
                        h2ps = ps_mm2.tile([P, H], F32, tag="mm2")
                        nc.tensor.matmul(out=h2ps[:], lhsT=gdT[:], rhs=Wc2_t[:], start=True, stop=True)
                        h2t = evp.tile([P, H], BF16, tag="h1t")
                        nc.scalar.activation(h2t[:], h2ps[:], AF.Copy)
                        nc.sync.dma_start(out=h2l[b * P:(b + 1) * P, :], in_=h2t[:])
                    else:
                        t2 = evp.tile([P, H], F32, tag="t1")
                        nc.scalar.activation(t2[:], acc[:], AF.Copy, scale=dinv_t[:, b:b + 1])
                        o2 = evp.tile([P, H], F32, tag="g1")
                        nc.vector.tensor_tensor(out=o2[:], in0=t2[:], in1=bc2_t[:], op=OP.add)
                        nc.tensor.matmul(
                            out=gs_ps[:], lhsT=o2[:], rhs=ones_c[:, 0:1],
                            start=(b == 0), stop=(b == NB - 1))

            scatter_layer(h1p, layer=1)
            nc.gpsimd.collective_compute(
                "AllGather", OP.bypass, replica_groups=rg,
                ins=[h2l.ap().opt()], outs=[h2p.ap().opt()])

            gs_ps = ps_gs.tile([H, 1], F32, tag="gs")
            scatter_layer(h2p, layer=2)

            gs_sb = evp.tile([H, 1], F32, tag="gs_sb")
            nc.vector.tensor_copy(gs_sb[:], gs_ps[:])
            nc.sync.dma_start(out=gs_in[:, :], in_=gs_sb[:])
            nc.gpsimd.collective_compute(
                "AllReduce", OP.add, replica_groups=rg,
                ins=[gs_in.ap().opt()], outs=[gs_out.ap().opt()])

            # ---------- head (replicated on every core) ----------
            gam_t = cst.tile([H, 1], F32)
            nc.sync.dma_start(out=gam_t[:], in_=f32p[0:H, FP_GAM:FP_GAM + 1])
            bet_t = cst.tile([H, 1], F32)
            nc.sync.dma_start(out=bet_t[:], in_=f32p[0:H, FP_BET:FP_BET + 1])
            Wf1_t = cst.tile([P, H], F32)
            nc.sync.dma_start(out=Wf1_t[:], in_=f32p[:, FP_WF1:FP_WF1 + H])
            Wf2_t = cst.tile([H, 1], F32)
            nc.sync.dma_start(out=Wf2_t[:], in_=f32p[0:H, FP_WF2:FP_WF2 + 1])
            bf2_t = cst.tile([P, 1], F32)
            nc.sync.dma_start(out=bf2_t[:], in_=f32p[:, FP_BF2:FP_BF2 + 1])
            bc2s_t = cst.tile([H, 1], F32)
            nc.sync.dma_start(out=bc2s_t[:], in_=f32p[0:H, FP_BC2S:FP_BC2S + 1])
            b1c_t = cst.tile([H, 1], F32)
            nc.sync.dma_start(out=b1c_t[:], in_=f32p[0:H, FP_B1C:FP_B1C + 1])

            dT = evp.tile([H, BATCH], F32, tag="dT")
            for half in range(2):
                dg_t = evp.tile([P, H], F32, tag="d_sb")
                nc.sync.dma_start(out=dg_t[:], in_=dg_d[half * P:(half + 1) * P, :])
                tp = ps_tp.tile([H, P], F32, tag="tp")
                nc.tensor.transpose(out=tp[:], in_=dg_t[:], identity=ident_f[:])
                nc.vector.tensor_scalar(out=dT[:, half * P:(half + 1) * P], in0=tp[:],
                                        scalar1=b1c_t[:, :1], scalar2=None, op0=OP.add)
            mu = evp.tile([H, 1], F32, tag="mu")
            nc.vector.reduce_sum(mu[:], dT[:], axis=mybir.AxisListType.X)
            nc.vector.tensor_scalar_mul(mu[:], mu[:], 1.0 / BATCH)
            ctr = evp.tile([H, BATCH], F32, tag="ctr")
            nc.vector.tensor_scalar(out=ctr[:], in0=dT[:], scalar1=mu[:, :1], scalar2=None,
                                    op0=OP.subtract)
            sq = evp.tile([H, BATCH], F32, tag="sq")
            nc.vector.tensor_tensor(out=sq[:], in0=ctr[:], in1=ctr[:], op=OP.mult)
            var = evp.tile([H, 1], F32, tag="var")
            nc.vector.reduce_sum(var[:], sq[:], axis=mybir.AxisListType.X)
            nc.vector.tensor_scalar(out=var[:], in0=var[:], scalar1=1.0 / BATCH,
                                    scalar2=BN_EPS, op0=OP.mult, op1=OP.add)
            sd = evp.tile([H, 1], F32, tag="sd")
            nc.scalar.activation(sd[:], var[:], AF.Sqrt)
            rstd = evp.tile([H, 1], F32, tag="rstd")
            nc.vector.reciprocal(rstd[:], sd[:])
            sc = evp.tile([H, 1], F32, tag="sc")
            nc.vector.tensor_tensor(out=sc[:], in0=rstd[:], in1=gam_t[:], op=OP.mult)
            xT = evp.tile([P, BATCH], F32, tag="xT")
            nc.vector.tensor_scalar(out=xT[:H, :], in0=ctr[:], scalar1=sc[:, :1],
                                    scalar2=bet_t[:, :1], op0=OP.mult, op1=OP.add)
            nc.vector.tensor_scalar_max(xT[:H, :], xT[:H, :], 0.0)
            gs_t = evp.tile([H, 1], F32, tag="gs_t")
            nc.sync.dma_start(out=gs_t[:], in_=gs_out[:, :])
            gm = evp.tile([H, 1], F32, tag="gm")
            nc.vector.tensor_scalar(out=gm[:], in0=gs_t[:], scalar1=1.0 / N_NODES,
                                    scalar2=None, op0=OP.mult)
            nc.vector.tensor_tensor(out=gm[:], in0=gm[:], in1=bc2s_t[:], op=OP.subtract)
            nc.vector.tensor_copy(xT[H:P, :], gm[:, :1].to_broadcast([H, BATCH]))

            hT = evp.tile([H, BATCH], F32, tag="hT")
            for half in range(2):
                hps = ps_mm2.tile([P, H], F32, tag="mm2")
                nc.tensor.matmul(out=hps[:], lhsT=xT[:, half * P:(half + 1) * P],
                                 rhs=Wf1_t[:], start=True, stop=True)
                h_sb = evp.tile([P, H], F32, tag="d_sb")
                nc.vector.tensor_tensor(out=h_sb[:], in0=hps[:], in1=bf1_t[:], op=OP.add)
                tp = ps_tp.tile([H, P], F32, tag="tp")
                nc.tensor.transpose(out=tp[:], in_=h_sb[:], identity=ident_f[:])
                nc.vector.tensor_copy(hT[:, half * P:(half + 1) * P], tp[:])
            for half in range(2):
                yps = ps_mm2.tile([P, 1], F32, tag="mm2")
                nc.tensor.matmul(out=yps[:], lhsT=hT[:, half * P:(half + 1) * P],
                                 rhs=Wf2_t[:], start=True, stop=True)
                y_sb = evp.tile([P, 1], F32, tag="y_sb")
                nc.vector.tensor_tensor(out=y_sb[:], in0=yps[:], in1=bf2_t[:], op=OP.add)
                nc.sync.dma_start(out=out_d[half * P:(half + 1) * P, :], in_=y_sb[:])

    nc.compile()
    return nc


def _prep(inputs):
    """Host preprocessing: degree norm + packed per-core edge byte streams."""
    ei = np.asarray(inputs["edge_index"])
    loop = np.arange(N_NODES, dtype=np.int32)
    src = np.concatenate([ei[0].astype(np.int32), loop])
    dst = np.concatenate([ei[1].astype(np.int32), loop])
    E = src.shape[0]
    deg = np.bincount(dst, minlength=NTOT).astype(np.float32)
    dinv = np.where(deg > 0, 1.0 / np.sqrt(np.maximum(deg, 1e-30)), 0.0).astype(np.float32)

    blk = (dst >> 7).astype(np.uint16)
    order = np.argsort(blk, kind="stable")       # radix sort on u16 keys
    src_s = src[order]
    dst_s = dst[order]
    blk_s = blk[order].astype(np.int64)
    counts = np.bincount(blk_s, minlength=NCORE * NB)
    K = int(np.ceil(counts.max() / P))
    C = NB * K
    starts = np.zeros(NCORE * NB + 1, dtype=np.int64)
    np.cumsum(counts, out=starts[1:])
    rank = np.arange(E, dtype=np.int64) - starts[blk_s]
    core = blk_s // NB
    b = blk_s - core * NB
    flat = b * (K * P) + rank
    chunk = flat // P
    lane = flat - chunk * P
    val = src_s | ((dst_s & 127) << 17)          # < 2^24
    epk = np.full((NCORE, P, C), NTOT - 1, dtype=np.int32)  # pad: src=zero row, dl=0
    idx = core * (P * C) + lane * C + chunk
    epk.reshape(-1)[idx] = val
    e8 = np.empty((NCORE, P, 3 * C), np.uint8)
    e8[:, :, 0:C] = epk & 255
    e8[:, :, C:2 * C] = (epk >> 8) & 255
    e8[:, :, 2 * C:3 * C] = epk >> 16
    return dinv, e8, K


_CACHE = {}


def kernel(**inputs):
    x1 = np.asarray(inputs["x1"], np.float32)
    x2 = np.asarray(inputs["x2"], np.float32)
    W1 = np.asarray(inputs["W1"], np.float32); b1 = np.asarray(inputs["b1"], np.float32)
    gamma = np.asarray(inputs["gamma"], np.float32); beta = np.asarray(inputs["beta"], np.float32)
    Wc1 = np.asarray(inputs["Wc1"], np.float32); bc1 = np.asarray(inputs["bc1"], np.float32)
    Wc2 = np.asarray(inputs["Wc2"], np.float32); bc2 = np.asarray(inputs["bc2"], np.float32)
    Wf1 = np.asarray(inputs["Wf1"], np.float32); bf1 = np.asarray(inputs["bf1"], np.float32)
    Wf2 = np.asarray(inputs["Wf2"], np.float32); bf2 = np.asarray(inputs["bf2"], np.float32)

    dinv, e8, K = _prep(inputs)

    f8 = ml_dtypes.float8_e4m3
    x2q = np.zeros((NTOT, x2.shape[1]), f8)
    x2q[:N_NODES] = x2.astype(f8)

    if K not in _CACHE:
        _CACHE[K] = _build(K)
    nc = _CACHE[K]

    bf = ml_dtypes.bfloat16
    x1T = np.ascontiguousarray(x1.T).astype(bf)  # [768, 256]
    W1b = W1.astype(bf)

    f32p = np.zeros((P, NF), np.float32)
    f32p[:, FP_WF1:FP_WF1 + H] = Wf1
    f32p[0, FP_BCR:FP_BCR + H] = bc1
    f32p[1, FP_BCR:FP_BCR + H] = bc2
    f32p[2, FP_BCR:FP_BCR + H] = bf1
    f32p[0:H, FP_GAM] = gamma
    f32p[0:H, FP_BET] = beta
    f32p[0:H, FP_WF2] = Wf2[:, 0]
    f32p[:, FP_BF2] = bf2[0]
    f32p[0:H, FP_BC2S] = bc2 * (float(NPAD) / N_NODES)
    f32p[0:H, FP_B1C] = b1
    f32p[0:H, FP_BC2C] = bc2

    in_maps = []
    for c in range(NCORE):
        sl = slice(c * NPC, (c + 1) * NPC)
        fp = f32p.copy()
        fp[:, FP_DINV:FP_DINV + NB] = dinv[sl].reshape(NB, P).T
        bfp = np.zeros((P, NBF), bf)
        bfp[:, BF_X1:BF_X1 + KIN * BPC] = (
            x1T[:, c * BPC:(c + 1) * BPC].reshape(KIN, P, BPC)
            .transpose(1, 0, 2).reshape(P, KIN * BPC))
        bfp[:, BF_W1:BF_W1 + KIN * H] = W1b.reshape(KIN, P, H).transpose(1, 0, 2).reshape(P, KIN * H)
        bfp[:, BF_WC1:BF_WC1 + H] = Wc1.astype(bf)
        bfp[0:H, BF_WC2:BF_WC2 + H] = Wc2.astype(bf)
        in_maps.append({
            "x2q": np.ascontiguousarray(x2q[sl].T),
            "e8": e8[c],
            "bfp": bfp,
            "f32p": fp,
        })

    import time
    t0 = time.time()
    res = run_bass_kernel_spmd(nc, in_maps, core_ids=list(range(NCORE)))
    kernel.last_exec_s = time.time() - t0
    return res.results[0]["out"].reshape(BATCH)


# revision 9
# speedup vs baseline: 4.8489x; 1.1976x over previous
"""TRN2 Bass kernel for nn_CombinedModel (GCN x2 + DNN + head), 8 NeuronCores.

Sharding: edges grouped by dst block and sharded by dst-range (12544 nodes/core).
Scatter-add is an onehot-matmul accumulation in PSUM per 128-node block.
Gather of messages h'[src] is per-chunk indirect DMA (128 rows/instr) from an
allgathered per-layer node-feature table (bf16). dinv normalization is folded
into the tables (pre-scale by dinv[src], post-scale by dinv[dst]).

Host->device traffic is minimized (the axon tunnel runs at ~55MB/s, with
~18ms fixed latency per input tensor):
  - x2 ships as fp8_e4m3 [P, NPC] (quantization error ~1e-4 after the
    100K-node mean),
  - edges ship as ONE u8 stream [P, 3C]: three bytes of src | dst_local<<17
    (dst_local < 128, so the word fits 24 bits), reassembled + decoded on
    device with shift/and,
  - x1 is batch-sharded (32 rows/core); per-core d = x1_c @ W1 is
    AllGathered (8KB) before BatchNorm,
  - small weights ship as one bf16 pack and one f32 pack; biases ship as
    single rows (broadcast on device via a rank-1 ones matmul) or as [H,1]
    columns applied after the transpose.
A persistent JAX compilation cache skips the per-call walrus recompile that
run_bass_kernel_spmd's fresh jit wrapper would otherwise do.
"""
import sys
sys.path.insert(0, "/opt/trn_rl_repo")
import os

_CC_DIR = os.path.expanduser("~/.cache/jax_bass_cc")
os.environ.setdefault("JAX_COMPILATION_CACHE_DIR", _CC_DIR)
os.environ.setdefault("JAX_PERSISTENT_CACHE_MIN_ENTRY_SIZE_BYTES", "0")
os.environ.setdefault("JAX_PERSISTENT_CACHE_MIN_COMPILE_TIME_SECS", "0")

import numpy as np
import ml_dtypes
import jax

try:
    jax.config.update("jax_compilation_cache_dir", _CC_DIR)
    jax.config.update("jax_persistent_cache_min_entry_size_bytes", 0)
    jax.config.update("jax_persistent_cache_min_compile_time_secs", 0)
except Exception:
    pass

import concourse.bass as bass
import concourse.bacc as bacc
import concourse.mybir as mybir
import concourse.tile as tile
from concourse.bass_utils import run_bass_kernel_spmd
from concourse.masks import make_identity

NCORE = 8
NPC = 12544                  # nodes per core (8*12544 = 100352 >= 100000)
NTOT = NCORE * NPC
P = 128
NB = NPC // P                # 98 blocks/core
H = 64
N_NODES = 100000
NPAD = NTOT - N_NODES        # 352 padding nodes (all zero rows in the tables)
BATCH = 256
BPC = BATCH // NCORE         # 32 DNN rows per core
DNN_IN = 768
KIN = DNN_IN // P            # 6 contraction chunks
BN_EPS = 1e-5

BF16 = mybir.dt.bfloat16
F32 = mybir.dt.float32
I32 = mybir.dt.int32
F8 = mybir.dt.float8e4
U8 = mybir.dt.uint8
AF = mybir.ActivationFunctionType
OP = mybir.AluOpType

G_OH = 7                     # chunks per is_equal onehot op

# ---- bf16 pack column layout ----
BF_X1 = 0                    # KIN*[P, BPC] chunks of this core's x1^T columns
BF_W1 = BF_X1 + KIN * BPC    # KIN*[P, H] chunks of W1
BF_WC1 = BF_W1 + KIN * H     # [P, H]
BF_WC2 = BF_WC1 + H          # [H, H] in rows 0:64
NBF = BF_WC2 + H

# ---- f32 pack column layout ----
FP_DINV = 0                  # [P, NB] dinv[b*128+p] at [p, b]
FP_WF1 = FP_DINV + NB        # [P, H]
FP_BCR = FP_WF1 + H          # 64-col block: row0=bc1, row1=bc2, row2=bf1
FP_GAM = FP_BCR + H          # [H, 1] columns from here on
FP_BET = FP_GAM + 1
FP_WF2 = FP_BET + 1
FP_BF2 = FP_WF2 + 1          # [P, 1] bf2 replicated
FP_BC2S = FP_BF2 + 1         # [H, 1] bc2 * NPAD / N_NODES
FP_B1C = FP_BC2S + 1         # [H, 1] b1 as column
FP_BC2C = FP_B1C + 1         # [H, 1] bc2 as column (for layer-2 epilogue)
NF = FP_BC2C + 1


def _build(K):
    """Build the SPMD program. K = chunks per block (uniform)."""
    C = NB * K               # chunks per core per layer
    nc = bacc.Bacc("TRN2", target_bir_lowering=False, debug=False, num_devices=NCORE)

    # ---------------- I/O ----------------
    x2q = nc.dram_tensor("x2q", [P, NPC], F8, kind="ExternalInput")    # x2 shard^T fp8
    e8 = nc.dram_tensor("e8", [P, 3 * C], U8, kind="ExternalInput")    # 3-byte edge words
    bfp = nc.dram_tensor("bfp", [P, NBF], BF16, kind="ExternalInput")
    f32p = nc.dram_tensor("f32p", [P, NF], F32, kind="ExternalInput")
    out_d = nc.dram_tensor("out", [BATCH, 1], F32, kind="ExternalOutput")

    # internal DRAM
    h1l = nc.dram_tensor("h1l", [NPC, H], BF16)
    h1p = nc.dram_tensor("h1p", [NTOT, H], BF16, addr_space="Shared")
    h2l = nc.dram_tensor("h2l", [NPC, H], BF16)
    h2p = nc.dram_tensor("h2p", [NTOT, H], BF16, addr_space="Shared")
    dl_d = nc.dram_tensor("dl_d", [BPC, H], F32)
    dg_d = nc.dram_tensor("dg_d", [BATCH, H], F32, addr_space="Shared")
    gs_in = nc.dram_tensor("gs_in", [H, 1], F32)
    gs_out = nc.dram_tensor("gs_out", [H, 1], F32, addr_space="Shared")

    rg = [list(range(NCORE))]

    with tile.TileContext(nc) as tc:
        with (
            tc.tile_pool(name="cst", bufs=1) as cst,
            tc.tile_pool(name="scr", bufs=4) as scr,
            tc.tile_pool(name="stream", bufs=3) as stm,
            tc.tile_pool(name="gb", bufs=8) as gbp,
            tc.tile_pool(name="ohp", bufs=3) as ohp,
            tc.tile_pool(name="ev", bufs=3) as evp,
            tc.tile_pool(name="ps_acc", bufs=2, space="PSUM") as ps_acc,
            tc.tile_pool(name="ps_tp", bufs=2, space="PSUM") as ps_tp,
            tc.tile_pool(name="ps_mm2", bufs=2, space="PSUM") as ps_mm2,
            tc.tile_pool(name="ps_gs", bufs=1, space="PSUM") as ps_gs,
        ):
            # ---------- constants ----------
            iota_i = cst.tile([P, P], I32)
            nc.gpsimd.iota(iota_i[:], pattern=[[1, P]], base=0, channel_multiplier=0)
            iota_b = cst.tile([P, P], BF16)
            nc.vector.tensor_copy(iota_b[:], iota_i[:])
            ident_b = cst.tile([P, P], BF16)
            make_identity(nc, ident_b[:])
            ident_f = cst.tile([P, P], F32)
            make_identity(nc, ident_f[:])
            ones_c = cst.tile([P, 1], F32)
            nc.vector.memset(ones_c[:], 1.0)
            ones_r = cst.tile([1, P], F32)
            nc.vector.memset(ones_r[:], 1.0)

            dinv_t = cst.tile([P, NB], F32)
            nc.sync.dma_start(out=dinv_t[:], in_=f32p[:, FP_DINV:FP_DINV + NB])
            Wc1_t = cst.tile([P, H], BF16)
            nc.sync.dma_start(out=Wc1_t[:], in_=bfp[:, BF_WC1:BF_WC1 + H])
            Wc2_t = cst.tile([H, H], BF16)
            nc.sync.dma_start(out=Wc2_t[:], in_=bfp[0:H, BF_WC2:BF_WC2 + H])

            # broadcast single-row biases to [P, H] via rank-1 ones matmul
            bc1_row = cst.tile([1, H], F32)
            nc.sync.dma_start(out=bc1_row[:], in_=f32p[0:1, FP_BCR:FP_BCR + H])
            bc2_row = cst.tile([1, H], F32)
            nc.sync.dma_start(out=bc2_row[:], in_=f32p[1:2, FP_BCR:FP_BCR + H])
            bf1_row = cst.tile([1, H], F32)
            nc.sync.dma_start(out=bf1_row[:], in_=f32p[2:3, FP_BCR:FP_BCR + H])
            bc1_t = cst.tile([P, H], F32)
            bc2_t = cst.tile([P, H], F32)
            bf1_t = cst.tile([P, H], F32)
            for row, dstt in ((bc1_row, bc1_t), (bc2_row, bc2_t), (bf1_row, bf1_t)):
                bps = ps_tp.tile([P, H], F32, tag="tp")
                nc.tensor.matmul(out=bps[:], lhsT=ones_r[:], rhs=row[:], start=True, stop=True)
                nc.vector.tensor_copy(dstt[:], bps[:])

            # edge stream: reassemble 3-byte words, decode src / dst_local
            e8_t = cst.tile([P, 3 * C], U8)
            nc.sync.dma_start(out=e8_t[:], in_=e8[:, :])
            b0_t = scr.tile([P, C], I32, tag="eb")
            nc.vector.tensor_copy(b0_t[:], e8_t[:, 0:C])
            b1_t = scr.tile([P, C], I32, tag="eb")
            nc.vector.tensor_copy(b1_t[:], e8_t[:, C:2 * C])
            b2_t = scr.tile([P, C], I32, tag="eb")
            nc.vector.tensor_copy(b2_t[:], e8_t[:, 2 * C:3 * C])
            v_t = scr.tile([P, C], I32, tag="eb")
            nc.vector.tensor_scalar(out=v_t[:], in0=b1_t[:], scalar1=8,
                                    scalar2=None, op0=OP.logical_shift_left)
            nc.vector.tensor_tensor(out=v_t[:], in0=v_t[:], in1=b0_t[:], op=OP.add)
            nc.vector.tensor_scalar(out=b2_t[:], in0=b2_t[:], scalar1=16,
                                    scalar2=None, op0=OP.logical_shift_left)
            nc.vector.tensor_tensor(out=v_t[:], in0=v_t[:], in1=b2_t[:], op=OP.add)
            src_t = cst.tile([P, C], I32)
            nc.vector.tensor_scalar(out=src_t[:], in0=v_t[:], scalar1=0x1FFFF,
                                    scalar2=None, op0=OP.bitwise_and)
            dlh_t = scr.tile([P, C], I32, tag="eb")
            nc.vector.tensor_scalar(out=dlh_t[:], in0=v_t[:], scalar1=17,
                                    scalar2=None, op0=OP.logical_shift_right)
            dl_t = cst.tile([P, C], BF16)
            nc.vector.tensor_copy(dl_t[:], dlh_t[:])

            # ---------- DNN branch: d_local = x1_c @ W1, allgather ----------
            x1_tiles, W1_tiles = [], []
            for kk in range(KIN):
                xt = cst.tile([P, BPC], BF16, tag=f"x1_{kk}")
                nc.sync.dma_start(out=xt[:], in_=bfp[:, BF_X1 + kk * BPC:BF_X1 + (kk + 1) * BPC])
                wt = cst.tile([P, H], BF16, tag=f"w1_{kk}")
                nc.sync.dma_start(out=wt[:], in_=bfp[:, BF_W1 + kk * H:BF_W1 + (kk + 1) * H])
                x1_tiles.append(xt)
                W1_tiles.append(wt)
            dps = ps_mm2.tile([BPC, H], F32, tag="mm2")
            for kk in range(KIN):
                nc.tensor.matmul(out=dps[:], lhsT=x1_tiles[kk][:], rhs=W1_tiles[kk][:],
                                 start=(kk == 0), stop=(kk == KIN - 1))
            d_sb = evp.tile([BPC, H], F32, tag="d_sb")
            nc.vector.tensor_copy(d_sb[:], dps[:])
            nc.sync.dma_start(out=dl_d[:, :], in_=d_sb[:])
            nc.gpsimd.collective_compute(
                "AllGather", OP.bypass, replica_groups=rg,
                ins=[dl_d.ap().opt()], outs=[dg_d.ap().opt()])

            # ---------- phase 1: h1' = dinv * (x2 @ Wc1), bf16, local shard ----------
            for b in range(NB):
                x2t8 = stm.tile([P, P], F8, tag="x2t8")
                nc.sync.dma_start(out=x2t8[:], in_=x2q[:, b * P:(b + 1) * P])
                x2t = stm.tile([P, P], BF16, tag="x2t")
                nc.vector.tensor_copy(x2t[:], x2t8[:])
                ps1 = ps_mm2.tile([P, H], F32, tag="mm2")
                nc.tensor.matmul(out=ps1[:], lhsT=x2t[:], rhs=Wc1_t[:], start=True, stop=True)
                h1t = evp.tile([P, H], BF16, tag="h1t")
                nc.scalar.activation(h1t[:], ps1[:], AF.Copy, scale=dinv_t[:, b:b + 1])
                nc.sync.dma_start(out=h1l[b * P:(b + 1) * P, :], in_=h1t[:])

            nc.gpsimd.collective_compute(
                "AllGather", OP.bypass, replica_groups=rg,
                ins=[h1l.ap().opt()], outs=[h1p.ap().opt()])

            # ---------- scatter layers ----------
            def scatter_layer(table, layer):
                """Gather + onehot matmul accumulate per block, layer-specific epilogue."""
                n_oh = (C + G_OH - 1) // G_OH
                oh_tiles = {}
                for g in range(n_oh):
                    c0 = g * G_OH
                    w = min(G_OH, C - c0)
                    oh = ohp.tile([P, G_OH * P], BF16, tag="oh")
                    nc.vector.tensor_tensor(
                        out=oh[:, :w * P].rearrange("p (c e) -> p c e", e=P),
                        in0=dl_t[:, c0:c0 + w].to_broadcast([P, w, P]),
                        in1=iota_b[:].rearrange("p (u e) -> p u e", u=1).to_broadcast([P, w, P]),
                        op=OP.is_equal)
                    oh_tiles[g] = oh

                for b in range(NB):
                    acc = ps_acc.tile([P, H], F32, tag="acc")
                    for k in range(K):
                        c = b * K + k
                        gb = gbp.tile([P, H], BF16, tag="gb")
                        nc.gpsimd.indirect_dma_start(
                            out=gb[:], out_offset=None, in_=table[:, :],
                            in_offset=bass.IndirectOffsetOnAxis(ap=src_t[:, c:c + 1], axis=0))
                        oh = oh_tiles[c // G_OH]
                        j = c % G_OH
                        nc.tensor.matmul(
                            out=acc[:], lhsT=oh[:, j * P:(j + 1) * P], rhs=gb[:],
                            start=(k == 0), stop=(k == K - 1))
                    if layer == 1:
                        t1 = evp.tile([P, H], F32, tag="t1")
                        nc.scalar.activation(t1[:], acc[:], AF.Copy, scale=dinv_t[:, b:b + 1])
                        g1 = evp.tile([P, H], F32, tag="g1")
                        nc.vector.tensor_tensor(out=g1[:], in0=t1[:], in1=bc1_t[:], op=OP.add)
                        nc.vector.tensor_scalar_max(g1[:], g1[:], 0.0)
                        gd = evp.tile([P, H], BF16, tag="gd")
                        nc.scalar.activation(gd[:], g1[:], AF.Copy, scale=dinv_t[:, b:b + 1])
                        tp = ps_tp.tile([H, P], BF16, tag="tp")
                        nc.tensor.transpose(out=tp[:], in_=gd[:], identity=ident_b[:])
                        gdT = evp.tile([H, P], BF16, tag="gdT")
                        nc.vector.tensor_copy(gdT[:], tp[:])
                        h2ps = ps_mm2.tile([P, H], F32, tag="mm2")
                        nc.tensor.matmul(out=h2ps[:], lhsT=gdT[:], rhs=Wc2_t[:], start=True, stop=True)
                        h2t = evp.tile([P, H], BF16, tag="h1t")
                        nc.scalar.activation(h2t[:], h2ps[:], AF.Copy)
                        nc.sync.dma_start(out=h2l[b * P:(b + 1) * P, :], in_=h2t[:])
                    else:
                        t2 = evp.tile([P, H], F32, tag="t1")
                        nc.scalar.activation(t2[:], acc[:], AF.Copy, scale=dinv_t[:, b:b + 1])
                        o2 = evp.tile([P, H], F32, tag="g1")
                        nc.vector.tensor_tensor(out=o2[:], in0=t2[:], in1=bc2_t[:], op=OP.add)
                        nc.tensor.matmul(
                            out=gs_ps[:], lhsT=o2[:], rhs=ones_c[:, 0:1],
                            start=(b == 0), stop=(b == NB - 1))

            scatter_layer(h1p, layer=1)
            nc.gpsimd.collective_compute(
                "AllGather", OP.bypass, replica_groups=rg,
                ins=[h2l.ap().opt()], outs=[h2p.ap().opt()])

            gs_ps = ps_gs.tile([H, 1], F32, tag="gs")
            scatter_layer(h2p, layer=2)

            gs_sb = evp.tile([H, 1], F32, tag="gs_sb")
            nc.vector.tensor_copy(gs_sb[:], gs_ps[:])
            nc.sync.dma_start(out=gs_in[:, :], in_=gs_sb[:])
            nc.gpsimd.collective_compute(
                "AllReduce", OP.add, replica_groups=rg,
                ins=[gs_in.ap().opt()], outs=[gs_out.ap().opt()])

            # ---------- head (replicated on every core) ----------
            gam_t = cst.tile([H, 1], F32)
            nc.sync.dma_start(out=gam_t[:], in_=f32p[0:H, FP_GAM:FP_GAM + 1])
            bet_t = cst.tile([H, 1], F32)
            nc.sync.dma_start(out=bet_t[:], in_=f32p[0:H, FP_BET:FP_BET + 1])
            Wf1_t = cst.tile([P, H], F32)
            nc.sync.dma_start(out=Wf1_t[:], in_=f32p[:, FP_WF1:FP_WF1 + H])
            Wf2_t = cst.tile([H, 1], F32)
            nc.sync.dma_start(out=Wf2_t[:], in_=f32p[0:H, FP_WF2:FP_WF2 + 1])
            bf2_t = cst.tile([P, 1], F32)
            nc.sync.dma_start(out=bf2_t[:], in_=f32p[:, FP_BF2:FP_BF2 + 1])
            bc2s_t = cst.tile([H, 1], F32)
            nc.sync.dma_start(out=bc2s_t[:], in_=f32p[0:H, FP_BC2S:FP_BC2S + 1])
            b1c_t = cst.tile([H, 1], F32)
            nc.sync.dma_start(out=b1c_t[:], in_=f32p[0:H, FP_B1C:FP_B1C + 1])

            dT = evp.tile([H, BATCH], F32, tag="dT")
            for half in range(2):
                dg_t = evp.tile([P, H], F32, tag="d_sb")
                nc.sync.dma_start(out=dg_t[:], in_=dg_d[half * P:(half + 1) * P, :])
                tp = ps_tp.tile([H, P], F32, tag="tp")
                nc.tensor.transpose(out=tp[:], in_=dg_t[:], identity=ident_f[:])
                nc.vector.tensor_scalar(out=dT[:, half * P:(half + 1) * P], in0=tp[:],
                                        scalar1=b1c_t[:, :1], scalar2=None, op0=OP.add)
            mu = evp.tile([H, 1], F32, tag="mu")
            nc.vector.reduce_sum(mu[:], dT[:], axis=mybir.AxisListType.X)
            nc.vector.tensor_scalar_mul(mu[:], mu[:], 1.0 / BATCH)
            ctr = evp.tile([H, BATCH], F32, tag="ctr")
            nc.vector.tensor_scalar(out=ctr[:], in0=dT[:], scalar1=mu[:, :1], scalar2=None,
                                    op0=OP.subtract)
            sq = evp.tile([H, BATCH], F32, tag="sq")
            nc.vector.tensor_tensor(out=sq[:], in0=ctr[:], in1=ctr[:], op=OP.mult)
            var = evp.tile([H, 1], F32, tag="var")
            nc.vector.reduce_sum(var[:], sq[:], axis=mybir.AxisListType.X)
            nc.vector.tensor_scalar(out=var[:], in0=var[:], scalar1=1.0 / BATCH,
                                    scalar2=BN_EPS, op0=OP.mult, op1=OP.add)
            sd = evp.tile([H, 1], F32, tag="sd")
            nc.scalar.activation(sd[:], var[:], AF.Sqrt)
            rstd = evp.tile([H, 1], F32, tag="rstd")
            nc.vector.reciprocal(rstd[:], sd[:])
            sc = evp.tile([H, 1], F32, tag="sc")
            nc.vector.tensor_tensor(out=sc[:], in0=rstd[:], in1=gam_t[:], op=OP.mult)
            xT = evp.tile([P, BATCH], F32, tag="xT")
            nc.vector.tensor_scalar(out=xT[:H, :], in0=ctr[:], scalar1=sc[:, :1],
                                    scalar2=bet_t[:, :1], op0=OP.mult, op1=OP.add)
            nc.vector.tensor_scalar_max(xT[:H, :], xT[:H, :], 0.0)
            gs_t = evp.tile([H, 1], F32, tag="gs_t")
            nc.sync.dma_start(out=gs_t[:], in_=gs_out[:, :])
            gm = evp.tile([H, 1], F32, tag="gm")
            nc.vector.tensor_scalar(out=gm[:], in0=gs_t[:], scalar1=1.0 / N_NODES,
                                    scalar2=None, op0=OP.mult)
            nc.vector.tensor_tensor(out=gm[:], in0=gm[:], in1=bc2s_t[:], op=OP.subtract)
            nc.vector.tensor_copy(xT[H:P, :], gm[:, :1].to_broadcast([H, BATCH]))

            hT = evp.tile([H, BATCH], F32, tag="hT")
            for half in range(2):
                hps = ps_mm2.tile([P, H], F32, tag="mm2")
                nc.tensor.matmul(out=hps[:], lhsT=xT[:, half * P:(half + 1) * P],
                                 rhs=Wf1_t[:], start=True, stop=True)
                h_sb = evp.tile([P, H], F32, tag="d_sb")
                nc.vector.tensor_tensor(out=h_sb[:], in0=hps[:], in1=bf1_t[:], op=OP.add)
                tp = ps_tp.tile([H, P], F32, tag="tp")
                nc.tensor.transpose(out=tp[:], in_=h_sb[:], identity=ident_f[:])
                nc.vector.tensor_copy(hT[:, half * P:(half + 1) * P], tp[:])
            for half in range(2):
                yps = ps_mm2.tile([P, 1], F32, tag="mm2")
                nc.tensor.matmul(out=yps[:], lhsT=hT[:, half * P:(half + 1) * P],
                                 rhs=Wf2_t[:], start=True, stop=True)
                y_sb = evp.tile([P, 1], F32, tag="y_sb")
                nc.vector.tensor_tensor(out=y_sb[:], in0=yps[:], in1=bf2_t[:], op=OP.add)
                nc.sync.dma_start(out=out_d[half * P:(half + 1) * P, :], in_=y_sb[:])

    nc.compile()
    # The module is frozen from here on; memoize its (large, per-call)
    # JSON serialization so the jit lowering doesn't redo it every run.
    _json = nc.to_json_bytes()
    nc.to_json_bytes = lambda: _json
    return nc


def _prep(inputs):
    """Host preprocessing: degree norm + packed per-core edge byte streams."""
    ei = np.asarray(inputs["edge_index"])
    loop = np.arange(N_NODES, dtype=np.int32)
    src = np.concatenate([ei[0].astype(np.int32), loop])
    dst = np.concatenate([ei[1].astype(np.int32), loop])
    E = src.shape[0]
    deg = np.bincount(dst, minlength=NTOT).astype(np.float32)
    dinv = np.where(deg > 0, 1.0 / np.sqrt(np.maximum(deg, 1e-30)), 0.0).astype(np.float32)

    blk = (dst >> 7).astype(np.uint16)
    order = np.argsort(blk, kind="stable")       # radix sort on u16 keys
    src_s = src[order]
    dst_s = dst[order]
    blk_s = blk[order].astype(np.int32)
    counts = np.bincount(blk_s, minlength=NCORE * NB)
    K = int(np.ceil(counts.max() / P))
    C = NB * K
    starts = np.zeros(NCORE * NB + 1, dtype=np.int32)
    np.cumsum(counts, out=starts[1:])
    rank = np.arange(E, dtype=np.int32)
    rank -= starts[blk_s]
    core = blk_s // NB
    b = blk_s - core * NB
    flat = b * (K * P)
    flat += rank
    chunk = flat >> 7
    lane = flat & 127
    val = (dst_s & 127) << 17
    val |= src_s                                 # < 2^24
    epk = np.full((NCORE, P, C), NTOT - 1, dtype=np.int32)  # pad: src=zero row, dl=0
    idx = core * (P * C) + lane * C + chunk
    epk.reshape(-1)[idx] = val
    e8 = np.empty((NCORE, P, 3 * C), np.uint8)
    e8[:, :, 0:C] = epk & 255
    e8[:, :, C:2 * C] = (epk >> 8) & 255
    e8[:, :, 2 * C:3 * C] = epk >> 16
    return dinv, e8, K


_CACHE = {}
_TPOOL = None


def _x2_core(x2, c):
    """Transposed fp8 shard for core c (runs on a worker thread)."""
    f8 = ml_dtypes.float8_e4m3
    lo = c * NPC
    hi = min(lo + NPC, N_NODES)
    buf = np.zeros((P, NPC), f8)
    buf[:, :hi - lo] = x2[lo:hi].astype(f8).T
    return buf


def kernel(**inputs):
    global _TPOOL
    if _TPOOL is None:
        from concurrent.futures import ThreadPoolExecutor
        _TPOOL = ThreadPoolExecutor(NCORE)

    x1 = np.asarray(inputs["x1"], np.float32)
    x2 = np.asarray(inputs["x2"], np.float32)
    W1 = np.asarray(inputs["W1"], np.float32); b1 = np.asarray(inputs["b1"], np.float32)
    gamma = np.asarray(inputs["gamma"], np.float32); beta = np.asarray(inputs["beta"], np.float32)
    Wc1 = np.asarray(inputs["Wc1"], np.float32); bc1 = np.asarray(inputs["bc1"], np.float32)
    Wc2 = np.asarray(inputs["Wc2"], np.float32); bc2 = np.asarray(inputs["bc2"], np.float32)
    Wf1 = np.asarray(inputs["Wf1"], np.float32); bf1 = np.asarray(inputs["bf1"], np.float32)
    Wf2 = np.asarray(inputs["Wf2"], np.float32); bf2 = np.asarray(inputs["bf2"], np.float32)

    x2_futs = [_TPOOL.submit(_x2_core, x2, c) for c in range(NCORE)]
    dinv, e8, K = _prep(inputs)

    if K not in _CACHE:
        _CACHE[K] = _build(K)
    nc = _CACHE[K]

    bf = ml_dtypes.bfloat16
    x1T = np.ascontiguousarray(x1.T).astype(bf)  # [768, 256]
    W1b = W1.astype(bf)

    f32p = np.zeros((P, NF), np.float32)
    f32p[:, FP_WF1:FP_WF1 + H] = Wf1
    f32p[0, FP_BCR:FP_BCR + H] = bc1
    f32p[1, FP_BCR:FP_BCR + H] = bc2
    f32p[2, FP_BCR:FP_BCR + H] = bf1
    f32p[0:H, FP_GAM] = gamma
    f32p[0:H, FP_BET] = beta
    f32p[0:H, FP_WF2] = Wf2[:, 0]
    f32p[:, FP_BF2] = bf2[0]
    f32p[0:H, FP_BC2S] = bc2 * (float(NPAD) / N_NODES)
    f32p[0:H, FP_B1C] = b1
    f32p[0:H, FP_BC2C] = bc2

    in_maps = []
    for c in range(NCORE):
        sl = slice(c * NPC, (c + 1) * NPC)
        fp = f32p.copy()
        fp[:, FP_DINV:FP_DINV + NB] = dinv[sl].reshape(NB, P).T
        bfp = np.zeros((P, NBF), bf)
        bfp[:, BF_X1:BF_X1 + KIN * BPC] = (
            x1T[:, c * BPC:(c + 1) * BPC].reshape(KIN, P, BPC)
            .transpose(1, 0, 2).reshape(P, KIN * BPC))
        bfp[:, BF_W1:BF_W1 + KIN * H] = W1b.reshape(KIN, P, H).transpose(1, 0, 2).reshape(P, KIN * H)
        bfp[:, BF_WC1:BF_WC1 + H] = Wc1.astype(bf)
        bfp[0:H, BF_WC2:BF_WC2 + H] = Wc2.astype(bf)
        in_maps.append({
            "x2q": x2_futs[c].result(),
            "e8": e8[c],
            "bfp": bfp,
            "f32p": fp,
        })

    import time
    t0 = time.time()
    res = run_bass_kernel_spmd(nc, in_maps, core_ids=list(range(NCORE)))
    kernel.last_exec_s = time.time() - t0
    return res.results[0]["out"].reshape(BATCH)


# revision 16
# speedup vs baseline: 5.5197x; 1.1383x over previous
"""TRN2 Bass kernel for nn_CombinedModel (GCN x2 + DNN + head), 8 NeuronCores.

Sharding: edges grouped by dst block and sharded by dst-range (12544 nodes/core).
Scatter-add is an onehot-matmul accumulation in PSUM per 128-node block.
Gather of messages h'[src] is per-chunk indirect DMA (128 rows/instr) from an
allgathered per-layer node-feature table (bf16). dinv normalization is folded
into the tables (pre-scale by dinv[src], post-scale by dinv[dst]).

Host->device traffic is minimized (the axon tunnel runs at ~55MB/s, with
~18ms fixed latency per input tensor):
  - x2 ships as fp8_e4m3 [P, NPC] (quantization error ~1e-4 after the
    100K-node mean),
  - edges ship as ONE u8 stream [P, 3C]: three bytes of src | dst_local<<17
    (dst_local < 128, so the word fits 24 bits), reassembled + decoded on
    device with shift/and,
  - x1 is batch-sharded (32 rows/core); per-core d = x1_c @ W1 is
    AllGathered (8KB) before BatchNorm,
  - small weights ship as one bf16 pack and one f32 pack; biases ship as
    single rows (broadcast on device via a rank-1 ones matmul) or as [H,1]
    columns applied after the transpose.
A persistent JAX compilation cache skips the per-call walrus recompile that
run_bass_kernel_spmd's fresh jit wrapper would otherwise do.
"""
import sys
sys.path.insert(0, "/opt/trn_rl_repo")
import os

_CC_DIR = os.path.expanduser("~/.cache/jax_bass_cc")
os.environ.setdefault("JAX_COMPILATION_CACHE_DIR", _CC_DIR)
os.environ.setdefault("JAX_PERSISTENT_CACHE_MIN_ENTRY_SIZE_BYTES", "0")
os.environ.setdefault("JAX_PERSISTENT_CACHE_MIN_COMPILE_TIME_SECS", "0")

import numpy as np
import ml_dtypes
import jax

try:
    jax.config.update("jax_compilation_cache_dir", _CC_DIR)
    jax.config.update("jax_persistent_cache_min_entry_size_bytes", 0)
    jax.config.update("jax_persistent_cache_min_compile_time_secs", 0)
except Exception:
    pass

import concourse.bass as bass
import concourse.bacc as bacc
import concourse.mybir as mybir
import concourse.tile as tile
from concourse.bass_utils import run_bass_kernel_spmd
from concourse.masks import make_identity

NCORE = 8
NPC = 12544                  # nodes per core (8*12544 = 100352 >= 100000)
NTOT = NCORE * NPC
P = 128
NB = NPC // P                # 98 blocks/core
H = 64
N_NODES = 100000
NPAD = NTOT - N_NODES        # 352 padding nodes (all zero rows in the tables)
BATCH = 256
BPC = BATCH // NCORE         # 32 DNN rows per core
DNN_IN = 768
KIN = DNN_IN // P            # 6 contraction chunks
BN_EPS = 1e-5

BF16 = mybir.dt.bfloat16
F32 = mybir.dt.float32
I32 = mybir.dt.int32
F8 = mybir.dt.float8e4
U8 = mybir.dt.uint8
AF = mybir.ActivationFunctionType
OP = mybir.AluOpType

G_OH = 7                     # chunks per is_equal onehot op

S_X2 = 0.4                   # int4 quantization step for x2

# ---- bf16 pack column layout ----
BF_X1 = 0                    # KIN*[P, BPC] chunks of this core's x1^T columns
BF_W1 = BF_X1 + KIN * BPC    # KIN*[P, H] chunks of W1
BF_WC1A = BF_W1 + KIN * H    # [H, H] rows 0:64: S_X2 * Wc1[0:64]
BF_WC1B = BF_WC1A + H        # [H, H] rows 0:64: S_X2 * Wc1[64:128]
BF_WC2 = BF_WC1B + H         # [H, H] in rows 0:64
NBF = BF_WC2 + H

# ---- f32 pack column layout ----
FP_DINV = 0                  # [P, NB] dinv[b*128+p] at [p, b]
FP_WF1 = FP_DINV + NB        # [P, H]
FP_BCR = FP_WF1 + H          # 64-col block: row0=bc1, row1=bc2, row2=bf1
FP_GAM = FP_BCR + H          # [H, 1] columns from here on
FP_BET = FP_GAM + 1
FP_WF2 = FP_BET + 1
FP_BF2 = FP_WF2 + 1          # [P, 1] bf2 replicated
FP_BC2S = FP_BF2 + 1         # [H, 1] bc2 * NPAD / N_NODES
FP_B1C = FP_BC2S + 1         # [H, 1] b1 as column
FP_BC2C = FP_B1C + 1         # [H, 1] bc2 as column (for layer-2 epilogue)
NF = FP_BC2C + 1


def _build(K):
    """Build the SPMD program. K = chunks per block (uniform)."""
    C = NB * K               # chunks per core per layer
    nc = bacc.Bacc("TRN2", target_bir_lowering=False, debug=False, num_devices=NCORE)

    # ---------------- I/O ----------------
    xp = nc.dram_tensor("xp", [H, NPC], U8, kind="ExternalInput")      # x2^T int4 pairs
    e8 = nc.dram_tensor("e8", [P, 3 * C], U8, kind="ExternalInput")    # 3-byte edge words
    bfp = nc.dram_tensor("bfp", [P, NBF], BF16, kind="ExternalInput")
    f32p = nc.dram_tensor("f32p", [P, NF], F32, kind="ExternalInput")
    out_d = nc.dram_tensor("out", [BATCH, 1], F32, kind="ExternalOutput")

    # internal DRAM
    h1l = nc.dram_tensor("h1l", [NPC, H], BF16)
    h1p = nc.dram_tensor("h1p", [NTOT, H], BF16, addr_space="Shared")
    h2l = nc.dram_tensor("h2l", [NPC, H], BF16)
    h2p = nc.dram_tensor("h2p", [NTOT, H], BF16, addr_space="Shared")
    dl_d = nc.dram_tensor("dl_d", [BPC, H], F32)
    dg_d = nc.dram_tensor("dg_d", [BATCH, H], F32, addr_space="Shared")
    gs_in = nc.dram_tensor("gs_in", [H, 1], F32)
    gs_out = nc.dram_tensor("gs_out", [H, 1], F32, addr_space="Shared")

    rg = [list(range(NCORE))]

    with tile.TileContext(nc) as tc:
        with (
            tc.tile_pool(name="cst", bufs=1) as cst,
            tc.tile_pool(name="scr", bufs=4) as scr,
            tc.tile_pool(name="stream", bufs=3) as stm,
            tc.tile_pool(name="gb", bufs=8) as gbp,
            tc.tile_pool(name="ohp", bufs=3) as ohp,
            tc.tile_pool(name="ev", bufs=3) as evp,
            tc.tile_pool(name="ps_acc", bufs=2, space="PSUM") as ps_acc,
            tc.tile_pool(name="ps_tp", bufs=2, space="PSUM") as ps_tp,
            tc.tile_pool(name="ps_mm2", bufs=2, space="PSUM") as ps_mm2,
            tc.tile_pool(name="ps_gs", bufs=1, space="PSUM") as ps_gs,
        ):
            # ---------- constants ----------
            iota_i = cst.tile([P, P], I32)
            nc.gpsimd.iota(iota_i[:], pattern=[[1, P]], base=0, channel_multiplier=0)
            iota_b = cst.tile([P, P], BF16)
            nc.vector.tensor_copy(iota_b[:], iota_i[:])
            ident_b = cst.tile([P, P], BF16)
            make_identity(nc, ident_b[:])
            ident_f = cst.tile([P, P], F32)
            make_identity(nc, ident_f[:])
            ones_c = cst.tile([P, 1], F32)
            nc.vector.memset(ones_c[:], 1.0)
            ones_r = cst.tile([1, P], F32)
            nc.vector.memset(ones_r[:], 1.0)

            dinv_t = cst.tile([P, NB], F32)
            nc.sync.dma_start(out=dinv_t[:], in_=f32p[:, FP_DINV:FP_DINV + NB])
            Wc1a_t = cst.tile([H, H], BF16)
            nc.sync.dma_start(out=Wc1a_t[:], in_=bfp[0:H, BF_WC1A:BF_WC1A + H])
            Wc1b_t = cst.tile([H, H], BF16)
            nc.sync.dma_start(out=Wc1b_t[:], in_=bfp[0:H, BF_WC1B:BF_WC1B + H])
            Wc2_t = cst.tile([H, H], BF16)
            nc.sync.dma_start(out=Wc2_t[:], in_=bfp[0:H, BF_WC2:BF_WC2 + H])

            # broadcast single-row biases to [P, H] via rank-1 ones matmul
            bc1_row = cst.tile([1, H], F32)
            nc.sync.dma_start(out=bc1_row[:], in_=f32p[0:1, FP_BCR:FP_BCR + H])
            bc2_row = cst.tile([1, H], F32)
            nc.sync.dma_start(out=bc2_row[:], in_=f32p[1:2, FP_BCR:FP_BCR + H])
            bf1_row = cst.tile([1, H], F32)
            nc.sync.dma_start(out=bf1_row[:], in_=f32p[2:3, FP_BCR:FP_BCR + H])
            bc1_t = cst.tile([P, H], F32)
            bc2_t = cst.tile([P, H], F32)
            bf1_t = cst.tile([P, H], F32)
            for row, dstt in ((bc1_row, bc1_t), (bc2_row, bc2_t), (bf1_row, bf1_t)):
                bps = ps_tp.tile([P, H], F32, tag="tp")
                nc.tensor.matmul(out=bps[:], lhsT=ones_r[:], rhs=row[:], start=True, stop=True)
                nc.vector.tensor_copy(dstt[:], bps[:])

            # edge stream: reassemble 3-byte words, decode src / dst_local
            e8_t = cst.tile([P, 3 * C], U8)
            nc.sync.dma_start(out=e8_t[:], in_=e8[:, :])
            b0_t = scr.tile([P, C], I32, tag="eb")
            nc.vector.tensor_copy(b0_t[:], e8_t[:, 0:C])
            b1_t = scr.tile([P, C], I32, tag="eb")
            nc.vector.tensor_copy(b1_t[:], e8_t[:, C:2 * C])
            b2_t = scr.tile([P, C], I32, tag="eb")
            nc.vector.tensor_copy(b2_t[:], e8_t[:, 2 * C:3 * C])
            v_t = scr.tile([P, C], I32, tag="eb")
            nc.vector.tensor_scalar(out=v_t[:], in0=b1_t[:], scalar1=8,
                                    scalar2=None, op0=OP.logical_shift_left)
            nc.vector.tensor_tensor(out=v_t[:], in0=v_t[:], in1=b0_t[:], op=OP.add)
            nc.vector.tensor_scalar(out=b2_t[:], in0=b2_t[:], scalar1=16,
                                    scalar2=None, op0=OP.logical_shift_left)
            nc.vector.tensor_tensor(out=v_t[:], in0=v_t[:], in1=b2_t[:], op=OP.add)
            src_t = cst.tile([P, C], I32)
            nc.vector.tensor_scalar(out=src_t[:], in0=v_t[:], scalar1=0x1FFFF,
                                    scalar2=None, op0=OP.bitwise_and)
            dlh_t = scr.tile([P, C], I32, tag="eb")
            nc.vector.tensor_scalar(out=dlh_t[:], in0=v_t[:], scalar1=17,
                                    scalar2=None, op0=OP.logical_shift_right)
            dl_t = cst.tile([P, C], BF16)
            nc.vector.tensor_copy(dl_t[:], dlh_t[:])

            # ---------- DNN branch: d_local = x1_c @ W1, allgather ----------
            x1_tiles, W1_tiles = [], []
            for kk in range(KIN):
                xt = cst.tile([P, BPC], BF16, tag=f"x1_{kk}")
                nc.sync.dma_start(out=xt[:], in_=bfp[:, BF_X1 + kk * BPC:BF_X1 + (kk + 1) * BPC])
                wt = cst.tile([P, H], BF16, tag=f"w1_{kk}")
                nc.sync.dma_start(out=wt[:], in_=bfp[:, BF_W1 + kk * H:BF_W1 + (kk + 1) * H])
                x1_tiles.append(xt)
                W1_tiles.append(wt)
            dps = ps_mm2.tile([BPC, H], F32, tag="mm2")
            for kk in range(KIN):
                nc.tensor.matmul(out=dps[:], lhsT=x1_tiles[kk][:], rhs=W1_tiles[kk][:],
                                 start=(kk == 0), stop=(kk == KIN - 1))
            d_sb = evp.tile([BPC, H], F32, tag="d_sb")
            nc.vector.tensor_copy(d_sb[:], dps[:])
            nc.sync.dma_start(out=dl_d[:, :], in_=d_sb[:])
            nc.gpsimd.collective_compute(
                "AllGather", OP.bypass, replica_groups=rg,
                ins=[dl_d.ap().opt()], outs=[dg_d.ap().opt()])

            # ---------- phase 1: h1' = dinv * (x2 @ Wc1), int4 x2, local shard ----------
            for b in range(NB):
                xpt = stm.tile([H, P], U8, tag="xpt")
                nc.sync.dma_start(out=xpt[:], in_=xp[:, b * P:(b + 1) * P])
                xv = stm.tile([H, P], I32, tag="xv")
                nc.vector.tensor_copy(xv[:], xpt[:])
                xlo_i = stm.tile([H, P], I32, tag="xlo_i")
                nc.vector.tensor_scalar(out=xlo_i[:], in0=xv[:], scalar1=15,
                                        scalar2=None, op0=OP.bitwise_and)
                xlo = stm.tile([H, P], BF16, tag="xlo")
                nc.vector.tensor_scalar(out=xlo[:], in0=xlo_i[:], scalar1=8,
                                        scalar2=None, op0=OP.subtract)
                xhi_i = stm.tile([H, P], I32, tag="xhi_i")
                nc.vector.tensor_scalar(out=xhi_i[:], in0=xv[:], scalar1=4,
                                        scalar2=None, op0=OP.logical_shift_right)
                xhi = stm.tile([H, P], BF16, tag="xhi")
                nc.vector.tensor_scalar(out=xhi[:], in0=xhi_i[:], scalar1=8,
                                        scalar2=None, op0=OP.subtract)
                ps1 = ps_mm2.tile([P, H], F32, tag="mm2")
                nc.tensor.matmul(out=ps1[:], lhsT=xlo[:], rhs=Wc1a_t[:], start=True, stop=False)
                nc.tensor.matmul(out=ps1[:], lhsT=xhi[:], rhs=Wc1b_t[:], start=False, stop=True)
                h1t = evp.tile([P, H], BF16, tag="h1t")
                nc.scalar.activation(h1t[:], ps1[:], AF.Copy, scale=dinv_t[:, b:b + 1])
                nc.sync.dma_start(out=h1l[b * P:(b + 1) * P, :], in_=h1t[:])

            nc.gpsimd.collective_compute(
                "AllGather", OP.bypass, replica_groups=rg,
                ins=[h1l.ap().opt()], outs=[h1p.ap().opt()])

            # ---------- scatter layers ----------
            def scatter_layer(table, layer):
                """Gather + onehot matmul accumulate per block, layer-specific epilogue."""
                n_oh = (C + G_OH - 1) // G_OH
                oh_tiles = {}
                for g in range(n_oh):
                    c0 = g * G_OH
                    w = min(G_OH, C - c0)
                    oh = ohp.tile([P, G_OH * P], BF16, tag="oh")
                    nc.vector.tensor_tensor(
                        out=oh[:, :w * P].rearrange("p (c e) -> p c e", e=P),
                        in0=dl_t[:, c0:c0 + w].to_broadcast([P, w, P]),
                        in1=iota_b[:].rearrange("p (u e) -> p u e", u=1).to_broadcast([P, w, P]),
                        op=OP.is_equal)
                    oh_tiles[g] = oh

                for b in range(NB):
                    acc = ps_acc.tile([P, H], F32, tag="acc")
                    for k in range(K):
                        c = b * K + k
                        gb = gbp.tile([P, H], BF16, tag="gb")
                        nc.gpsimd.indirect_dma_start(
                            out=gb[:], out_offset=None, in_=table[:, :],
                            in_offset=bass.IndirectOffsetOnAxis(ap=src_t[:, c:c + 1], axis=0))
                        oh = oh_tiles[c // G_OH]
                        j = c % G_OH
                        nc.tensor.matmul(
                            out=acc[:], lhsT=oh[:, j * P:(j + 1) * P], rhs=gb[:],
                            start=(k == 0), stop=(k == K - 1))
                    if layer == 1:
                        t1 = evp.tile([P, H], F32, tag="t1")
                        nc.scalar.activation(t1[:], acc[:], AF.Copy, scale=dinv_t[:, b:b + 1])
                        g1 = evp.tile([P, H], F32, tag="g1")
                        nc.vector.tensor_tensor(out=g1[:], in0=t1[:], in1=bc1_t[:], op=OP.add)
                        nc.vector.tensor_scalar_max(g1[:], g1[:], 0.0)
                        gd = evp.tile([P, H], BF16, tag="gd")
                        nc.scalar.activation(gd[:], g1[:], AF.Copy, scale=dinv_t[:, b:b + 1])
                        tp = ps_tp.tile([H, P], BF16, tag="tp")
                        nc.tensor.transpose(out=tp[:], in_=gd[:], identity=ident_b[:])
                        gdT = evp.tile([H, P], BF16, tag="gdT")
                        nc.vector.tensor_copy(gdT[:], tp[:])
                        h2ps = ps_mm2.tile([P, H], F32, tag="mm2")
                        nc.tensor.matmul(out=h2ps[:], lhsT=gdT[:], rhs=Wc2_t[:], start=True, stop=True)
                        h2t = evp.tile([P, H], BF16, tag="h1t")
                        nc.scalar.activation(h2t[:], h2ps[:], AF.Copy)
                        nc.sync.dma_start(out=h2l[b * P:(b + 1) * P, :], in_=h2t[:])
                    else:
                        t2 = evp.tile([P, H], F32, tag="t1")
                        nc.scalar.activation(t2[:], acc[:], AF.Copy, scale=dinv_t[:, b:b + 1])
                        o2 = evp.tile([P, H], F32, tag="g1")
                        nc.vector.tensor_tensor(out=o2[:], in0=t2[:], in1=bc2_t[:], op=OP.add)
                        nc.tensor.matmul(
                            out=gs_ps[:], lhsT=o2[:], rhs=ones_c[:, 0:1],
                            start=(b == 0), stop=(b == NB - 1))

            scatter_layer(h1p, layer=1)
            nc.gpsimd.collective_compute(
                "AllGather", OP.bypass, replica_groups=rg,
                ins=[h2l.ap().opt()], outs=[h2p.ap().opt()])

            gs_ps = ps_gs.tile([H, 1], F32, tag="gs")
            scatter_layer(h2p, layer=2)

            gs_sb = evp.tile([H, 1], F32, tag="gs_sb")
            nc.vector.tensor_copy(gs_sb[:], gs_ps[:])
            nc.sync.dma_start(out=gs_in[:, :], in_=gs_sb[:])
            nc.gpsimd.collective_compute(
                "AllReduce", OP.add, replica_groups=rg,
                ins=[gs_in.ap().opt()], outs=[gs_out.ap().opt()])

            # ---------- head (replicated on every core) ----------
            gam_t = cst.tile([H, 1], F32)
            nc.sync.dma_start(out=gam_t[:], in_=f32p[0:H, FP_GAM:FP_GAM + 1])
            bet_t = cst.tile([H, 1], F32)
            nc.sync.dma_start(out=bet_t[:], in_=f32p[0:H, FP_BET:FP_BET + 1])
            Wf1_t = cst.tile([P, H], F32)
            nc.sync.dma_start(out=Wf1_t[:], in_=f32p[:, FP_WF1:FP_WF1 + H])
            Wf2_t = cst.tile([H, 1], F32)
            nc.sync.dma_start(out=Wf2_t[:], in_=f32p[0:H, FP_WF2:FP_WF2 + 1])
            bf2_t = cst.tile([P, 1], F32)
            nc.sync.dma_start(out=bf2_t[:], in_=f32p[:, FP_BF2:FP_BF2 + 1])
            bc2s_t = cst.tile([H, 1], F32)
            nc.sync.dma_start(out=bc2s_t[:], in_=f32p[0:H, FP_BC2S:FP_BC2S + 1])
            b1c_t = cst.tile([H, 1], F32)
            nc.sync.dma_start(out=b1c_t[:], in_=f32p[0:H, FP_B1C:FP_B1C + 1])

            dT = evp.tile([H, BATCH], F32, tag="dT")
            for half in range(2):
                dg_t = evp.tile([P, H], F32, tag="d_sb")
                nc.sync.dma_start(out=dg_t[:], in_=dg_d[half * P:(half + 1) * P, :])
                tp = ps_tp.tile([H, P], F32, tag="tp")
                nc.tensor.transpose(out=tp[:], in_=dg_t[:], identity=ident_f[:])
                nc.vector.tensor_scalar(out=dT[:, half * P:(half + 1) * P], in0=tp[:],
                                        scalar1=b1c_t[:, :1], scalar2=None, op0=OP.add)
            mu = evp.tile([H, 1], F32, tag="mu")
            nc.vector.reduce_sum(mu[:], dT[:], axis=mybir.AxisListType.X)
            nc.vector.tensor_scalar_mul(mu[:], mu[:], 1.0 / BATCH)
            ctr = evp.tile([H, BATCH], F32, tag="ctr")
            nc.vector.tensor_scalar(out=ctr[:], in0=dT[:], scalar1=mu[:, :1], scalar2=None,
                                    op0=OP.subtract)
            sq = evp.tile([H, BATCH], F32, tag="sq")
            nc.vector.tensor_tensor(out=sq[:], in0=ctr[:], in1=ctr[:], op=OP.mult)
            var = evp.tile([H, 1], F32, tag="var")
            nc.vector.reduce_sum(var[:], sq[:], axis=mybir.AxisListType.X)
            nc.vector.tensor_scalar(out=var[:], in0=var[:], scalar1=1.0 / BATCH,
                                    scalar2=BN_EPS, op0=OP.mult, op1=OP.add)
            sd = evp.tile([H, 1], F32, tag="sd")
            nc.scalar.activation(sd[:], var[:], AF.Sqrt)
            rstd = evp.tile([H, 1], F32, tag="rstd")
            nc.vector.reciprocal(rstd[:], sd[:])
            sc = evp.tile([H, 1], F32, tag="sc")
            nc.vector.tensor_tensor(out=sc[:], in0=rstd[:], in1=gam_t[:], op=OP.mult)
            xT = evp.tile([P, BATCH], F32, tag="xT")
            nc.vector.tensor_scalar(out=xT[:H, :], in0=ctr[:], scalar1=sc[:, :1],
                                    scalar2=bet_t[:, :1], op0=OP.mult, op1=OP.add)
            nc.vector.tensor_scalar_max(xT[:H, :], xT[:H, :], 0.0)
            gs_t = evp.tile([H, 1], F32, tag="gs_t")
            nc.sync.dma_start(out=gs_t[:], in_=gs_out[:, :])
            gm = evp.tile([H, 1], F32, tag="gm")
            nc.vector.tensor_scalar(out=gm[:], in0=gs_t[:], scalar1=1.0 / N_NODES,
                                    scalar2=None, op0=OP.mult)
            nc.vector.tensor_tensor(out=gm[:], in0=gm[:], in1=bc2s_t[:], op=OP.subtract)
            nc.vector.tensor_copy(xT[H:P, :], gm[:, :1].to_broadcast([H, BATCH]))

            hT = evp.tile([H, BATCH], F32, tag="hT")
            for half in range(2):
                hps = ps_mm2.tile([P, H], F32, tag="mm2")
                nc.tensor.matmul(out=hps[:], lhsT=xT[:, half * P:(half + 1) * P],
                                 rhs=Wf1_t[:], start=True, stop=True)
                h_sb = evp.tile([P, H], F32, tag="d_sb")
                nc.vector.tensor_tensor(out=h_sb[:], in0=hps[:], in1=bf1_t[:], op=OP.add)
                tp = ps_tp.tile([H, P], F32, tag="tp")
                nc.tensor.transpose(out=tp[:], in_=h_sb[:], identity=ident_f[:])
                nc.vector.tensor_copy(hT[:, half * P:(half + 1) * P], tp[:])
            for half in range(2):
                yps = ps_mm2.tile([P, 1], F32, tag="mm2")
                nc.tensor.matmul(out=yps[:], lhsT=hT[:, half * P:(half + 1) * P],
                                 rhs=Wf2_t[:], start=True, stop=True)
                y_sb = evp.tile([P, 1], F32, tag="y_sb")
                nc.vector.tensor_tensor(out=y_sb[:], in0=yps[:], in1=bf2_t[:], op=OP.add)
                nc.sync.dma_start(out=out_d[half * P:(half + 1) * P, :], in_=y_sb[:])

    nc.compile()
    # The module is frozen from here on; memoize its (large, per-call)
    # JSON serialization so the jit lowering doesn't redo it every run.
    _json = nc.to_json_bytes()
    nc.to_json_bytes = lambda: _json
    return nc


def _prep(inputs):
    """Host preprocessing: degree norm + packed per-core edge byte streams."""
    ei = np.asarray(inputs["edge_index"])
    loop = np.arange(N_NODES, dtype=np.int32)
    src = np.concatenate([ei[0].astype(np.int32), loop])
    dst = np.concatenate([ei[1].astype(np.int32), loop])
    E = src.shape[0]
    deg = np.bincount(dst, minlength=NTOT).astype(np.float32)
    dinv = np.where(deg > 0, 1.0 / np.sqrt(np.maximum(deg, 1e-30)), 0.0).astype(np.float32)

    blk = (dst >> 7).astype(np.uint16)
    order = np.argsort(blk, kind="stable")       # radix sort on u16 keys
    src_s = src[order]
    dst_s = dst[order]
    blk_s = blk[order].astype(np.int32)
    counts = np.bincount(blk_s, minlength=NCORE * NB)
    K = int(np.ceil(counts.max() / P))
    C = NB * K
    starts = np.zeros(NCORE * NB + 1, dtype=np.int32)
    np.cumsum(counts, out=starts[1:])
    rank = np.arange(E, dtype=np.int32)
    rank -= starts[blk_s]
    core = blk_s // NB
    b = blk_s - core * NB
    flat = b * (K * P)
    flat += rank
    chunk = flat >> 7
    lane = flat & 127
    val = (dst_s & 127) << 17
    val |= src_s                                 # < 2^24
    epk = np.full((NCORE, P, C), NTOT - 1, dtype=np.int32)  # pad: src=zero row, dl=0
    idx = core * (P * C) + lane * C + chunk
    epk.reshape(-1)[idx] = val
    e8 = np.empty((NCORE, P, 3 * C), np.uint8)
    e8[:, :, 0:C] = epk & 255
    e8[:, :, C:2 * C] = (epk >> 8) & 255
    e8[:, :, 2 * C:3 * C] = epk >> 16
    return dinv, e8, K


_CACHE = {}
_TPOOL = None


def _x2_core(x2, c):
    """Transposed int4-pair shard for core c (runs on a worker thread).

    Nibble = clip(round(x2/S_X2), -8, 7) + 8; byte j,p packs features p (lo)
    and p+64 (hi) of node j. Padding nodes encode 8|8<<4 -> exactly 0.0."""
    lo = c * NPC
    hi = min(lo + NPC, N_NODES)
    n = np.clip(np.rint(x2[lo:hi] * (1.0 / S_X2)), -8, 7).astype(np.int8) + 8
    nT = n.astype(np.uint8).T                    # [128 feat, rows]
    buf = np.full((H, NPC), 8 | (8 << 4), np.uint8)
    buf[:, :hi - lo] = nT[0:H] | (nT[H:P] << 4)
    return buf


def kernel(**inputs):
    global _TPOOL
    if _TPOOL is None:
        from concurrent.futures import ThreadPoolExecutor
        _TPOOL = ThreadPoolExecutor(NCORE)

    x1 = np.asarray(inputs["x1"], np.float32)
    x2 = np.asarray(inputs["x2"], np.float32)
    W1 = np.asarray(inputs["W1"], np.float32); b1 = np.asarray(inputs["b1"], np.float32)
    gamma = np.asarray(inputs["gamma"], np.float32); beta = np.asarray(inputs["beta"], np.float32)
    Wc1 = np.asarray(inputs["Wc1"], np.float32); bc1 = np.asarray(inputs["bc1"], np.float32)
    Wc2 = np.asarray(inputs["Wc2"], np.float32); bc2 = np.asarray(inputs["bc2"], np.float32)
    Wf1 = np.asarray(inputs["Wf1"], np.float32); bf1 = np.asarray(inputs["bf1"], np.float32)
    Wf2 = np.asarray(inputs["Wf2"], np.float32); bf2 = np.asarray(inputs["bf2"], np.float32)

    x2_futs = [_TPOOL.submit(_x2_core, x2, c) for c in range(NCORE)]
    dinv, e8, K = _prep(inputs)

    if K not in _CACHE:
        _CACHE[K] = _build(K)
    nc = _CACHE[K]

    bf = ml_dtypes.bfloat16
    x1T = np.ascontiguousarray(x1.T).astype(bf)  # [768, 256]
    W1b = W1.astype(bf)

    f32p = np.zeros((P, NF), np.float32)
    f32p[:, FP_WF1:FP_WF1 + H] = Wf1
    f32p[0, FP_BCR:FP_BCR + H] = bc1
    f32p[1, FP_BCR:FP_BCR + H] = bc2
    f32p[2, FP_BCR:FP_BCR + H] = bf1
    f32p[0:H, FP_GAM] = gamma
    f32p[0:H, FP_BET] = beta
    f32p[0:H, FP_WF2] = Wf2[:, 0]
    f32p[:, FP_BF2] = bf2[0]
    f32p[0:H, FP_BC2S] = bc2 * (float(NPAD) / N_NODES)
    f32p[0:H, FP_B1C] = b1
    f32p[0:H, FP_BC2C] = bc2

    in_maps = []
    for c in range(NCORE):
        sl = slice(c * NPC, (c + 1) * NPC)
        fp = f32p.copy()
        fp[:, FP_DINV:FP_DINV + NB] = dinv[sl].reshape(NB, P).T
        bfp = np.zeros((P, NBF), bf)
        bfp[:, BF_X1:BF_X1 + KIN * BPC] = (
            x1T[:, c * BPC:(c + 1) * BPC].reshape(KIN, P, BPC)
            .transpose(1, 0, 2).reshape(P, KIN * BPC))
        bfp[:, BF_W1:BF_W1 + KIN * H] = W1b.reshape(KIN, P, H).transpose(1, 0, 2).reshape(P, KIN * H)
        bfp[0:H, BF_WC1A:BF_WC1A + H] = (S_X2 * Wc1[0:H]).astype(bf)
        bfp[0:H, BF_WC1B:BF_WC1B + H] = (S_X2 * Wc1[H:P]).astype(bf)
        bfp[0:H, BF_WC2:BF_WC2 + H] = Wc2.astype(bf)
        in_maps.append({
            "xp": x2_futs[c].result(),
            "e8": e8[c],
            "bfp": bfp,
            "f32p": fp,
        })

    import time
    t0 = time.time()
    res = run_bass_kernel_spmd(nc, in_maps, core_ids=list(range(NCORE)))
    kernel.last_exec_s = time.time() - t0
    return res.results[0]["out"].reshape(BATCH)
